# revision 1
# baseline (speedup 1.0000x reference)
"""Trainium2 Bass kernel for CapsNet dynamic routing (ClassCapsules).

Reference computation (B=256, R=1152, C=10, O=16, I=8, 3 routing iters):
    u_hat[b,r,c,o] = sum_i W[r,c,o,i] * x[b,r,i]
    b_ij = 0
    for it in 3:
        c_ij = softmax(b_ij, axis=1)                      # over c
        s = sum_r c_ij[r,c] * u_hat[b,r,c,o] + bias       # [B,C,O]
        v = squash(s)
        if it < 2:
            b_ij += mean_b sum_o u_hat[b,r,c,o] v[b,c,o]  # [R,C]
    return v[..., None]

u_hat ([B,R,C,O] = 189MB fp32) is never materialized.  Both routing
contractions are re-associated through the factorization
    s[b,co]    = x~[b,(ri)] @ (c∘W~)[(ri),(co)]
    agree[r,c] = sum_{i,o} W~[(ri),(co)] * G[(ri),(co)],
                 G = (1/B) x~^T v
with x~ = x viewed as [B, R*I] and W~ = W viewed as [R*I, C*O].

Distribution: R is sharded 8 ways (144 r's per core).  Per iteration the
partial s ([256,160] fp32, 160KB) is summed across cores with one
AllReduce; the last iteration uses a ReduceScatter instead and each core
squashes + outputs its own 32-batch shard.  agree/b_ij/c_ij are fully
local to each core's r-shard.  An optional dependency-free warm-up
AllReduce at kernel start absorbs cross-core launch skew while inputs
load.
"""

import os
import sys
import types

sys.path.insert(0, "/opt/trn_rl_repo")

# Shim antenv.axon_hooks (absent on this image) so BASS_TRACE=1 profiling
# works through run_bass_kernel_spmd's axon path.  Harmless when unused.
try:
    import antenv.axon_hooks  # noqa: F401
except ImportError:
    try:
        _hooks = types.ModuleType("antenv.axon_hooks")
        _hooks._hook = None
        _hooks.set_axon_ntff_profile_hook = lambda h: setattr(_hooks, "_hook", h)
        _hooks.get_axon_ntff_profile_hook = lambda: _hooks._hook
        sys.modules["antenv.axon_hooks"] = _hooks
        import antenv
        antenv.axon_hooks = _hooks
        from trn_agent_boot.trn_boot import _ntff_profile_via_ctypes
        _hooks.set_axon_ntff_profile_hook(
            _ntff_profile_via_ctypes("/opt/axon/libaxon_pjrt.so")
        )
    except Exception:
        pass

import numpy as np

import concourse.bacc as bacc
import concourse.bass as bass
import concourse.tile as tile
from concourse import mybir
import concourse.bass_utils as _bass_utils
from concourse.bass_utils import run_bass_kernel_spmd

if os.environ.get("BASS_TRACE"):
    _bass_utils.upload_artifacts = lambda tmpdir: ""  # no bucket access here

LAST_RESULT = None

F32 = mybir.dt.float32
F16 = mybir.dt.float16
ALU = mybir.AluOpType
ACT = mybir.ActivationFunctionType

B, R, C, O, I = 256, 1152, 10, 16, 8
CO = C * O                      # 160
N_CORES = 8
R_LOC = R // N_CORES            # 144
RI_LOC = R_LOC * I              # 1152
NG = RI_LOC // 128              # 9 groups of 128 (r,i) rows
NB = B // 128                   # 2 batch partition chunks
B_SHARD = B // N_CORES          # 32 batches output per core
ITERS = 3
RPG = 128 // I                  # 16 r's per group

WARM_AR = os.environ.get("K_WARM_AR", "1") == "1"
CC_F16 = os.environ.get("K_CC_F16", "0") == "1"
CC_DT = F16 if CC_F16 else F32


def _squash(nc, eps_sb, t, n_part, nb, pool):
    """v = t * n2/((1+n2)*sqrt(n2+eps)); t: [n_part, nb, CO], reduce over o."""
    nc_ = nb * C
    tf = t.rearrange("p nb co -> p (nb co)")
    sq = pool.tile([n_part, nb * CO], F32, tag="sq")
    nc.vector.tensor_mul(sq, tf, tf)
    n2 = pool.tile([n_part, nc_], F32, tag="n2")
    nc.vector.reduce_sum(
        n2, sq.rearrange("p (nb c o) -> p nb c o", nb=nb, c=C),
        axis=mybir.AxisListType.X,
    )
    rt = pool.tile([n_part, nc_], F32, tag="rt")
    nc.scalar.activation(rt, n2, ACT.Sqrt, bias=eps_sb[:n_part])
    n2p1 = pool.tile([n_part, nc_], F32, tag="n2p1")
    nc.vector.tensor_scalar_add(n2p1, n2, 1.0)
    den = pool.tile([n_part, nc_], F32, tag="den")
    nc.vector.tensor_mul(den, n2p1, rt)
    rec = pool.tile([n_part, nc_], F32, tag="rec")
    nc.vector.reciprocal(rec, den)
    fac = pool.tile([n_part, nc_], F32, tag="fac")
    nc.vector.tensor_mul(fac, n2, rec)
    v = pool.tile([n_part, nb, CO], F32, tag="v")
    fac_b = fac.rearrange(
        "p (nb c one) -> p nb c one", nb=nb, c=C
    ).broadcast_to([n_part, nb, C, O])
    nc.vector.tensor_tensor(
        out=v.rearrange("p nb (c o) -> p nb c o", c=C),
        in0=t.rearrange("p nb (c o) -> p nb c o", c=C),
        in1=fac_b,
        op=ALU.mult,
    )
    return v


def build():
    nc = bacc.Bacc("TRN2", target_bir_lowering=False, debug=False,
                   num_devices=N_CORES)

    xt_d = nc.dram_tensor("xt", [RI_LOC, B], F32, kind="ExternalInput")
    xb_d = nc.dram_tensor("xb", [B, RI_LOC], F32, kind="ExternalInput")
    wg_d = nc.dram_tensor("wg", [RI_LOC, CO], F32, kind="ExternalInput")
    bias_d = nc.dram_tensor("biasf", [CO], F32, kind="ExternalInput")
    sel_d = nc.dram_tensor("sel", [128, RPG], F32, kind="ExternalInput")
    selT_d = nc.dram_tensor("selT", [RPG, 128], F32, kind="ExternalInput")
    y_d = nc.dram_tensor("y", [B_SHARD, CO], F32, kind="ExternalOutput")

    rg = [list(range(N_CORES))]

    with tile.TileContext(nc) as tc:
        with (
            tc.tile_pool(name="singles", bufs=1) as singles,
            tc.tile_pool(name="cw_pool", bufs=2) as cw_pool,
            tc.tile_pool(name="work", bufs=2) as work,
            tc.tile_pool(name="small", bufs=3) as small,
            tc.tile_pool(name="psum_s", bufs=1, space="PSUM") as psum_s,
            tc.tile_pool(name="psum_g", bufs=2, space="PSUM") as psum_g,
            tc.tile_pool(name="psum_misc", bufs=1, space="PSUM") as psum_misc,
            tc.tile_pool(name="dram", bufs=2, space="DRAM") as dram,
        ):
            if WARM_AR:
                warm_sb = singles.tile([1, 8], F32)
                nc.vector.memset(warm_sb, 0.0)
                warm_in = dram.tile([8], F32)
                warm_out = dram.tile([8], F32)
                nc.gpsimd.dma_start(out=warm_in[:], in_=warm_sb[0, :])
                nc.gpsimd.collective_compute(
                    "AllReduce", ALU.add, replica_groups=rg,
                    ins=[warm_in[:]], outs=[warm_out[:]],
                )

            # ---- load inputs ----
            XT = singles.tile([128, NG, B], F32)       # x~ [(ri),b] chunked
            nc.sync.dma_start(
                out=XT, in_=xt_d.ap().rearrange("(g p) b -> p g b", p=128)
            )
            XB = []                                    # x [b,(ri)] 2 p-chunks
            for kb in range(NB):
                t = singles.tile([128, RI_LOC], F32, tag=f"xb{kb}",
                                 name=f"xb_sb{kb}")
                nc.sync.dma_start(out=t, in_=xb_d[kb * 128:(kb + 1) * 128, :])
                XB.append(t)
            WG = singles.tile([128, NG, CO], F32)      # W~ [(ri),(co)] chunked
            nc.sync.dma_start(
                out=WG, in_=wg_d.ap().rearrange("(g p) n -> p g n", p=128)
            )
            biasb = singles.tile([128, CO], F32)
            nc.sync.dma_start(
                out=biasb,
                in_=bass.AP(tensor=bias_d, offset=0, ap=[[0, 128], [1, CO]]),
            )
            sel_sb = singles.tile([128, RPG], F32)
            nc.sync.dma_start(out=sel_sb, in_=sel_d[:, :])
            selT_sb = singles.tile([RPG, 128], F32)
            nc.sync.dma_start(out=selT_sb, in_=selT_d[:, :])

            eps_sb = singles.tile([128, 1], F32)
            nc.vector.memset(eps_sb, 1e-8)

            esr = None   # [16, 99]: exp(b_ij) ++ 1/sum_c exp(b_ij)

            for it in range(ITERS):
                # ---- CW = c∘W~ (it>0); it=0 uses uniform c=0.1 folded later
                if it == 0:
                    CW = WG
                else:
                    # cp_sb[:, :90] = e broadcast over i; [:, 90:99] = rec b.
                    cp_ps = psum_misc.tile([128, NG * C + NG], F32, tag="cp",
                                           name=f"cp_ps_{it}")
                    nc.tensor.matmul(cp_ps, selT_sb, esr, start=True, stop=True)
                    cp_sb = small.tile([128, NG * C + NG], F32, tag="cpart",
                                       name=f"cp_sb_{it}")
                    nc.scalar.copy(cp_sb, cp_ps)
                    CW = cw_pool.tile([128, NG, CO], F32, tag="cw",
                                      name=f"cw_{it}")
                    NGP = 6          # groups on DVE via stt; rest on gpsimd
                    cn = small.tile([128, (NG - NGP) * C], F32, tag="cn",
                                    name=f"cn_{it}")
                    rec_b3 = cp_sb[:, NG * C + NGP:NG * C + NG].rearrange(
                        "p (g one) -> p g one", one=1
                    ).broadcast_to([128, NG - NGP, C])
                    nc.vector.tensor_tensor(
                        out=cn.rearrange("p (g c) -> p g c", g=NG - NGP),
                        in0=cp_sb[:, NGP * C:NG * C].rearrange(
                            "p (g c) -> p g c", g=NG - NGP),
                        in1=rec_b3, op=ALU.mult,
                    )
                    for g in range(NG):
                        if g < NGP:
                            e_b = cp_sb[:, g * C:(g + 1) * C].rearrange(
                                "p (c one) -> p c one", one=1
                            ).broadcast_to([128, C, O])
                            nc.vector.scalar_tensor_tensor(
                                out=CW[:, g, :].rearrange(
                                    "p (c o) -> p c o", c=C),
                                in0=WG[:, g, :].rearrange(
                                    "p (c o) -> p c o", c=C),
                                scalar=cp_sb[:, NG * C + g:NG * C + g + 1],
                                in1=e_b,
                                op0=ALU.mult, op1=ALU.mult,
                            )
                        else:
                            c_b = cn[:, (g - NGP) * C:(g - NGP + 1) * C
                                     ].rearrange(
                                "p (c one) -> p c one", one=1
                            ).broadcast_to([128, C, O])
                            nc.gpsimd.tensor_tensor(
                                out=CW[:, g, :].rearrange(
                                    "p (c o) -> p c o", c=C),
                                in0=WG[:, g, :].rearrange(
                                    "p (c o) -> p c o", c=C),
                                in1=c_b, op=ALU.mult,
                            )

                # ---- s partial: [256,160] = x~^T @ CW, K = (ri) local ----
                s_ps = [psum_s.tile([128, CO], F32, tag=f"s{kb}",
                                    name=f"s_ps{kb}_{it}")
                        for kb in range(NB)]
                for kb in range(NB):
                    for g in range(NG):
                        nc.tensor.matmul(
                            s_ps[kb],
                            XT[:, g, kb * 128:(kb + 1) * 128],
                            CW[:, g, :],
                            start=(g == 0),
                            stop=(g == NG - 1),
                        )

                cc_in = dram.tile([NB, 128, CO], CC_DT, tag="cc_in",
                                  name=f"cc_in_{it}")
                for kb in range(NB):
                    s_stage = work.tile([128, CO], CC_DT, tag=f"sstage{kb}",
                                        name=f"s_stage{kb}_{it}")
                    nc.scalar.copy(s_stage, s_ps[kb])
                    nc.sync.dma_start(out=cc_in[kb, :, :], in_=s_stage)
                    if kb == NB - 1:
                        dsq = small.tile([1, 1], F32, tag="dsq",
                                         name=f"dsq_{it}")
                        nc.scalar.activation(dsq, s_stage[:1, :1], ACT.Sqrt,
                                             bias=eps_sb[:1], scale=0.0)

                if it < ITERS - 1:
                    # ---- AllReduce s; every core squashes the full batch
                    cc_out = dram.tile([NB, 128, CO], CC_DT, tag="cc_out",
                                       name=f"cc_out_{it}")
                    nc.gpsimd.collective_compute(
                        "AllReduce", ALU.add, replica_groups=rg,
                        ins=[cc_in.opt()], outs=[cc_out.opt()],
                    )
                    s_sb = work.tile([128, NB, CO], CC_DT, tag="ssb",
                                     name=f"s_sb_{it}")
                    for kb in range(NB):
                        nc.sync.dma_start(
                            out=s_sb[:, kb, :], in_=cc_out[kb, :, :]
                        )
                    warm_ps = psum_misc.tile([RPG, 512], F32, tag="warmps",
                                             name=f"warm_ps_{it}")
                    warm_rhs = XT[:, 0, :]            # [128, 256] static
                    for wi in range(14):
                        nc.tensor.matmul(
                            warm_ps[:, :B], sel_sb, warm_rhs,
                            start=(wi == 0), stop=True,
                            skip_group_check=True,
                        )
                    t = work.tile([128, NB, CO], F32, tag="t",
                                  name=f"t_{it}")
                    bias_b = biasb.rearrange(
                        "p (one co) -> p one co", one=1
                    ).broadcast_to([128, NB, CO])
                    nc.vector.scalar_tensor_tensor(
                        out=t, in0=s_sb,
                        scalar=(0.1 if it == 0 else 1.0),
                        in1=bias_b, op0=ALU.mult, op1=ALU.add,
                    )
                    v_sb = _squash(nc, eps_sb, t, 128, NB, work)

                    # ---- G = (1/B) x~^T v ; agree = sum_io W∘G ----
                    Q_all = small.tile([128, NG * C], F32, tag="qall",
                                       name=f"qall_{it}")
                    p9 = work.tile([128, NG, CO], F32, tag="p9",
                                   name=f"p9_{it}")
                    for g in range(NG):
                        g_ps = psum_g.tile([128, CO], F32, tag="gps",
                                           name=f"g_ps_{it}_{g}")
                        for kb in range(NB):
                            nc.tensor.matmul(
                                g_ps,
                                XB[kb][:, g * 128:(g + 1) * 128],
                                v_sb[:, kb, :],
                                start=(kb == 0),
                                stop=(kb == NB - 1),
                            )
                        nc.vector.scalar_tensor_tensor(
                            out=p9[:, g, :], in0=g_ps, scalar=1.0 / B,
                            in1=WG[:, g, :], op0=ALU.mult, op1=ALU.mult,
                        )
                        if g == 3 or g == 7 or g == 8:
                            lo = 0 if g == 3 else (4 if g == 7 else 8)
                            nc.vector.reduce_sum(
                                Q_all[:, lo * C:(g + 1) * C],
                                p9[:, lo:g + 1, :].rearrange(
                                    "p g (c o) -> p (g c) o", c=C),
                                axis=mybir.AxisListType.X,
                            )
                    agree_ps = psum_misc.tile([RPG, NG * C], F32, tag="agree",
                                              name=f"agree_{it}")
                    nc.tensor.matmul(agree_ps, sel_sb, Q_all,
                                     start=True, stop=True)

                    # ---- exp(b_ij) updated multiplicatively:
                    # exp(b_prev + agree) = exp(b_prev) * exp(agree) ----
                    esr_prev = esr
                    esr = small.tile([RPG, NG * C + NG], F32, tag="esr",
                                     name=f"esr_{it}")
                    if it == 0:
                        nc.scalar.activation(esr[:, :NG * C], agree_ps, ACT.Exp)
                    else:
                        eexp = small.tile([RPG, NG * C], F32, tag="eexp",
                                          name=f"eexp_{it}")
                        nc.scalar.activation(eexp, agree_ps, ACT.Exp)
                        nc.vector.tensor_mul(
                            esr[:, :NG * C], esr_prev[:, :NG * C], eexp
                        )
                    den = small.tile([RPG, NG], F32, tag="sden",
                                     name=f"den_{it}")
                    nc.vector.reduce_sum(
                        den,
                        esr[:, :NG * C].rearrange("p (g c) -> p g c", g=NG),
                        axis=mybir.AxisListType.X,
                    )
                    nc.vector.reciprocal(esr[:, NG * C:], den)
                else:
                    # ---- final iter: ReduceScatter; squash own b-shard ----
                    rs_out = dram.tile([B_SHARD * CO], CC_DT, tag="rs_out")
                    nc.gpsimd.collective_compute(
                        "ReduceScatter", ALU.add, replica_groups=rg,
                        ins=[cc_in.opt()], outs=[rs_out[:]],
                    )
                    s_sb = work.tile([B_SHARD, 1, CO], CC_DT, tag="fs")
                    nc.sync.dma_start(
                        out=s_sb,
                        in_=rs_out.rearrange("(p one n) -> p one n",
                                             n=CO, one=1),
                    )
                    t = work.tile([B_SHARD, 1, CO], F32, tag="ft")
                    bias_b1 = biasb[:B_SHARD, :].rearrange(
                        "p (one co) -> p one co", one=1
                    )
                    nc.vector.scalar_tensor_tensor(
                        out=t, in0=s_sb, scalar=1.0,
                        in1=bias_b1, op0=ALU.mult, op1=ALU.add,
                    )
                    v = _squash(nc, eps_sb, t, B_SHARD, 1, work)
                    nc.sync.dma_start(
                        out=y_d[:, :], in_=v.rearrange("p one co -> p (one co)")
                    )

    nc.compile()
    return nc


_NC = None


def kernel(x: np.ndarray, W: np.ndarray, bias: np.ndarray) -> np.ndarray:
    global _NC
    if _NC is None:
        _NC = build()

    x = np.ascontiguousarray(x, dtype=np.float32)
    W = np.ascontiguousarray(W, dtype=np.float32)
    bias = np.ascontiguousarray(bias, dtype=np.float32)

    biasf = bias.reshape(CO)
    sel = np.zeros((128, RPG), dtype=np.float32)
    sel[np.arange(128), np.arange(128) // I] = 1.0
    selT = np.ascontiguousarray(sel.T)

    in_maps = []
    for k in range(N_CORES):
        r0, r1 = k * R_LOC, (k + 1) * R_LOC
        xk = x[:, r0:r1, :].reshape(B, RI_LOC)          # [B,(r,i)]
        wk = W[r0:r1].transpose(0, 3, 1, 2).reshape(RI_LOC, CO)  # [(r,i),(c,o)]
        in_maps.append({
            "xt": np.ascontiguousarray(xk.T),
            "xb": np.ascontiguousarray(xk),
            "wg": np.ascontiguousarray(wk),
            "biasf": biasf,
            "sel": sel,
            "selT": selT,
        })

    global LAST_RESULT
    res = run_bass_kernel_spmd(
        _NC, in_maps, list(range(N_CORES)),
        trace=bool(os.environ.get("BASS_TRACE")),
    )
    LAST_RESULT = res
    v = np.concatenate([res.results[k]["y"] for k in range(N_CORES)], axis=0)
    return v.reshape(B, C, O)[..., None].astype(np.float32)



# revision 2
# speedup vs baseline: 1.1342x; 1.1342x over previous
"""Trainium2 Bass kernel for CapsNet dynamic routing (ClassCapsules).

Reference computation (B=256, R=1152, C=10, O=16, I=8, 3 routing iters):
    u_hat[b,r,c,o] = sum_i W[r,c,o,i] * x[b,r,i]
    b_ij = 0
    for it in 3:
        c_ij = softmax(b_ij, axis=1)                      # over c
        s = sum_r c_ij[r,c] * u_hat[b,r,c,o] + bias       # [B,C,O]
        v = squash(s)
        if it < 2:
            b_ij += mean_b sum_o u_hat[b,r,c,o] v[b,c,o]  # [R,C]
    return v[..., None]

u_hat ([B,R,C,O] = 189MB fp32) is never materialized.  Both routing
contractions are re-associated through the factorization
    s[b,co]    = x~[b,(ri)] @ (c∘W~)[(ri),(co)]
    agree[r,c] = sum_{i,o} W~[(ri),(co)] * G[(ri),(co)],
                 G = (1/B) x~^T v
with x~ = x viewed as [B, R*I] and W~ = W viewed as [R*I, C*O].

Distribution: R is sharded 8 ways (144 r's per core).  Per iteration the
partial s ([256,160], fp16 on the wire) is summed across cores with one
AllReduce; the last iteration uses a ReduceScatter instead and each core
squashes + outputs its own 32-batch shard.  agree/b_ij/c_ij are fully
local to each core's r-shard.

Matmul operands (x, W, c∘W, v) are fp16: 1 cycle/row on the PE (vs 4 for
fp32) and fast-weight-load; all accumulation stays fp32 in PSUM.
"""

import os
import sys
import types

sys.path.insert(0, "/opt/trn_rl_repo")

# Shim antenv.axon_hooks (absent on this image) so BASS_TRACE=1 profiling
# works through run_bass_kernel_spmd's axon path.  Harmless when unused.
try:
    import antenv.axon_hooks  # noqa: F401
except ImportError:
    try:
        _hooks = types.ModuleType("antenv.axon_hooks")
        _hooks._hook = None
        _hooks.set_axon_ntff_profile_hook = lambda h: setattr(_hooks, "_hook", h)
        _hooks.get_axon_ntff_profile_hook = lambda: _hooks._hook
        sys.modules["antenv.axon_hooks"] = _hooks
        import antenv
        antenv.axon_hooks = _hooks
        from trn_agent_boot.trn_boot import _ntff_profile_via_ctypes
        _hooks.set_axon_ntff_profile_hook(
            _ntff_profile_via_ctypes("/opt/axon/libaxon_pjrt.so")
        )
    except Exception:
        pass

import numpy as np

import concourse.bacc as bacc
import concourse.bass as bass
import concourse.tile as tile
from concourse import mybir
import concourse.bass_utils as _bass_utils
from concourse.bass_utils import run_bass_kernel_spmd

if os.environ.get("BASS_TRACE"):
    _bass_utils.upload_artifacts = lambda tmpdir: ""  # no bucket access here

LAST_RESULT = None

F32 = mybir.dt.float32
F16 = mybir.dt.float16
ALU = mybir.AluOpType
ACT = mybir.ActivationFunctionType

B, R, C, O, I = 256, 1152, 10, 16, 8
CO = C * O                      # 160
N_CORES = 8
R_LOC = R // N_CORES            # 144
RI_LOC = R_LOC * I              # 1152
NG = RI_LOC // 128              # 9 groups of 128 (r,i) rows
NB = B // 128                   # 2 batch partition chunks
B_SHARD = B // N_CORES          # 32 batches output per core
ITERS = 3
RPG = 128 // I                  # 16 r's per group

WARM_AR = os.environ.get("K_WARM_AR", "0") == "1"
CC_F16 = os.environ.get("K_CC_F16", "1") == "1"
MM_F16 = os.environ.get("K_MM_F16", "1") == "1"
CC_DT = F16 if CC_F16 else F32
MM_DT = F16 if MM_F16 else F32


def _squash(nc, eps_sb, t, n_part, nb, pool, tag, out_dt=F32):
    """v = t * n2/((1+n2)*sqrt(n2+eps)); t: [n_part, nb, CO], reduce over o."""
    nc_ = nb * C
    tf = t.rearrange("p nb co -> p (nb co)")
    sq = pool.tile([n_part, nb * CO], F32, tag=f"sq{tag}")
    nc.vector.tensor_mul(sq, tf, tf)
    n2 = pool.tile([n_part, nc_], F32, tag=f"n2{tag}")
    nc.vector.reduce_sum(
        n2, sq.rearrange("p (nb c o) -> p nb c o", nb=nb, c=C),
        axis=mybir.AxisListType.X,
    )
    rt = pool.tile([n_part, nc_], F32, tag=f"rt{tag}")
    nc.scalar.activation(rt, n2, ACT.Sqrt, bias=eps_sb[:n_part])
    n2p1 = pool.tile([n_part, nc_], F32, tag=f"n2p1{tag}")
    nc.vector.tensor_scalar_add(n2p1, n2, 1.0)
    den = pool.tile([n_part, nc_], F32, tag=f"den{tag}")
    nc.vector.tensor_mul(den, n2p1, rt)
    rec = pool.tile([n_part, nc_], F32, tag=f"rec{tag}")
    nc.vector.reciprocal(rec, den)
    fac = pool.tile([n_part, nc_], F32, tag=f"fac{tag}")
    nc.vector.tensor_mul(fac, n2, rec)
    v = pool.tile([n_part, nb, CO], out_dt, tag=f"v{tag}")
    fac_b = fac.rearrange(
        "p (nb c one) -> p nb c one", nb=nb, c=C
    ).broadcast_to([n_part, nb, C, O])
    nc.vector.tensor_tensor(
        out=v.rearrange("p nb (c o) -> p nb c o", c=C),
        in0=t.rearrange("p nb (c o) -> p nb c o", c=C),
        in1=fac_b,
        op=ALU.mult,
    )
    return v


def build():
    nc = bacc.Bacc("TRN2", target_bir_lowering=False, debug=False,
                   num_devices=N_CORES)

    xt_d = nc.dram_tensor("xt", [RI_LOC, B], MM_DT, kind="ExternalInput")
    xb_d = nc.dram_tensor("xb", [B, RI_LOC], MM_DT, kind="ExternalInput")
    wg_d = nc.dram_tensor("wg", [RI_LOC, CO], MM_DT, kind="ExternalInput")
    bias_d = nc.dram_tensor("biasf", [CO], F32, kind="ExternalInput")
    sel_d = nc.dram_tensor("sel", [128, RPG], F32, kind="ExternalInput")
    selT_d = nc.dram_tensor("selT", [RPG, 128], F32, kind="ExternalInput")
    y_d = nc.dram_tensor("y", [B_SHARD, CO], F32, kind="ExternalOutput")

    rg = [list(range(N_CORES))]

    with tile.TileContext(nc) as tc:
        with (
            tc.tile_pool(name="singles", bufs=1) as singles,
            tc.tile_pool(name="cw_pool", bufs=2) as cw_pool,
            tc.tile_pool(name="work", bufs=2) as work,
            tc.tile_pool(name="small", bufs=3) as small,
            tc.tile_pool(name="psum_s", bufs=1, space="PSUM") as psum_s,
            tc.tile_pool(name="psum_g", bufs=2, space="PSUM") as psum_g,
            tc.tile_pool(name="psum_misc", bufs=1, space="PSUM") as psum_misc,
            tc.tile_pool(name="dram", bufs=2, space="DRAM") as dram,
        ):
            if WARM_AR:
                warm_sb = singles.tile([1, 8], F32)
                nc.vector.memset(warm_sb, 0.0)
                warm_in = dram.tile([8], F32)
                warm_out = dram.tile([8], F32)
                nc.gpsimd.dma_start(out=warm_in[:], in_=warm_sb[0, :])
                nc.gpsimd.collective_compute(
                    "AllReduce", ALU.add, replica_groups=rg,
                    ins=[warm_in[:]], outs=[warm_out[:]],
                )

            # ---- load inputs (per-group DMAs so matmuls start early) ----
            XT = singles.tile([128, NG, B], MM_DT)     # x~ [(ri),b] chunked
            WG = singles.tile([128, NG, CO], MM_DT)    # W~ [(ri),(co)] chunked
            for g in range(NG):
                nc.sync.dma_start(out=XT[:, g, :],
                                  in_=xt_d[g * 128:(g + 1) * 128, :])
                nc.sync.dma_start(out=WG[:, g, :],
                                  in_=wg_d[g * 128:(g + 1) * 128, :])
            XB = []                                    # x [b,(ri)] 2 p-chunks
            for kb in range(NB):
                t = singles.tile([128, RI_LOC], MM_DT, tag=f"xb{kb}",
                                 name=f"xb_sb{kb}")
                nc.sync.dma_start(out=t, in_=xb_d[kb * 128:(kb + 1) * 128, :])
                XB.append(t)
            biasb = singles.tile([128, CO], F32)
            nc.sync.dma_start(
                out=biasb,
                in_=bass.AP(tensor=bias_d, offset=0, ap=[[0, 128], [1, CO]]),
            )
            sel_sb = singles.tile([128, RPG], F32)
            nc.sync.dma_start(out=sel_sb, in_=sel_d[:, :])
            selT_sb = singles.tile([RPG, 128], F32)
            nc.sync.dma_start(out=selT_sb, in_=selT_d[:, :])

            eps_sb = singles.tile([128, 1], F32)
            nc.vector.memset(eps_sb, 1e-8)

            esr = None   # [16, 99]: exp(b_ij) ++ 1/sum_c exp(b_ij)

            for it in range(ITERS):
                # ---- CW = c∘W~ (it>0); it=0 uses uniform c=0.1 folded later
                if it == 0:
                    CW = WG
                else:
                    # cp_sb[:, :90] = e broadcast over i; [:, 90:99] = rec b.
                    cp_ps = psum_misc.tile([128, NG * C + NG], F32, tag="cp",
                                           name=f"cp_ps_{it}")
                    nc.tensor.matmul(cp_ps, selT_sb, esr, start=True, stop=True)
                    cp_sb = small.tile([128, NG * C + NG], F32, tag="cpart",
                                       name=f"cp_sb_{it}")
                    nc.scalar.copy(cp_sb, cp_ps)
                    CW = cw_pool.tile([128, NG, CO], MM_DT, tag="cw",
                                      name=f"cw_{it}")
                    NGP = 6          # groups on DVE via stt; rest on gpsimd
                    cn = small.tile([128, (NG - NGP) * C], F32, tag="cn",
                                    name=f"cn_{it}")
                    rec_b3 = cp_sb[:, NG * C + NGP:NG * C + NG].rearrange(
                        "p (g one) -> p g one", one=1
                    ).broadcast_to([128, NG - NGP, C])
                    nc.vector.tensor_tensor(
                        out=cn.rearrange("p (g c) -> p g c", g=NG - NGP),
                        in0=cp_sb[:, NGP * C:NG * C].rearrange(
                            "p (g c) -> p g c", g=NG - NGP),
                        in1=rec_b3, op=ALU.mult,
                    )
                    for g in range(NG):
                        if g < NGP:
                            e_b = cp_sb[:, g * C:(g + 1) * C].rearrange(
                                "p (c one) -> p c one", one=1
                            ).broadcast_to([128, C, O])
                            nc.vector.scalar_tensor_tensor(
                                out=CW[:, g, :].rearrange(
                                    "p (c o) -> p c o", c=C),
                                in0=WG[:, g, :].rearrange(
                                    "p (c o) -> p c o", c=C),
                                scalar=cp_sb[:, NG * C + g:NG * C + g + 1],
                                in1=e_b,
                                op0=ALU.mult, op1=ALU.mult,
                            )
                        else:
                            c_b = cn[:, (g - NGP) * C:(g - NGP + 1) * C
                                     ].rearrange(
                                "p (c one) -> p c one", one=1
                            ).broadcast_to([128, C, O])
                            nc.gpsimd.tensor_tensor(
                                out=CW[:, g, :].rearrange(
                                    "p (c o) -> p c o", c=C),
                                in0=WG[:, g, :].rearrange(
                                    "p (c o) -> p c o", c=C),
                                in1=c_b, op=ALU.mult,
                            )

                # ---- s partial: [256,160] = x~^T @ CW, K = (ri) local ----
                s_ps = [psum_s.tile([128, CO], F32, tag=f"s{kb}",
                                    name=f"s_ps{kb}_{it}")
                        for kb in range(NB)]
                for kb in range(NB):
                    for g in range(NG):
                        nc.tensor.matmul(
                            s_ps[kb],
                            XT[:, g, kb * 128:(kb + 1) * 128],
                            CW[:, g, :],
                            start=(g == 0),
                            stop=(g == NG - 1),
                        )

                cc_in = dram.tile([NB, 128, CO], CC_DT, tag="cc_in",
                                  name=f"cc_in_{it}")
                for kb in range(NB):
                    s_stage = work.tile([128, CO], CC_DT, tag=f"sstage{kb}",
                                        name=f"s_stage{kb}_{it}")
                    nc.scalar.copy(s_stage, s_ps[kb])
                    nc.sync.dma_start(out=cc_in[kb, :, :], in_=s_stage)

                if it < ITERS - 1:
                    # ---- AllReduce s; every core squashes the full batch
                    cc_out = dram.tile([NB, 128, CO], CC_DT, tag="cc_out",
                                       name=f"cc_out_{it}")
                    nc.gpsimd.collective_compute(
                        "AllReduce", ALU.add, replica_groups=rg,
                        ins=[cc_in.opt()], outs=[cc_out.opt()],
                    )
                    # per-kb squash so the G matmuls for kb=0 overlap the
                    # squash of kb=1
                    v_sb = []
                    bias_b1 = biasb.rearrange(
                        "p (one co) -> p one co", one=1
                    )
                    for kb in range(NB):
                        s_kb = work.tile([128, 1, CO], CC_DT, tag=f"ssb{kb}",
                                         name=f"s_sb{kb}_{it}")
                        nc.sync.dma_start(
                            out=s_kb[:, 0, :], in_=cc_out[kb, :, :]
                        )
                        t = work.tile([128, 1, CO], F32, tag=f"t{kb}",
                                      name=f"t{kb}_{it}")
                        nc.vector.scalar_tensor_tensor(
                            out=t, in0=s_kb,
                            scalar=(0.1 if it == 0 else 1.0),
                            in1=bias_b1, op0=ALU.mult, op1=ALU.add,
                        )
                        v_sb.append(_squash(nc, eps_sb, t, 128, 1, work,
                                            tag=str(kb), out_dt=MM_DT))

                    # ---- G = (1/B) x~^T v ; agree = sum_io W∘G ----
                    Q_all = small.tile([128, NG * C], F32, tag="qall",
                                       name=f"qall_{it}")
                    p9 = work.tile([128, NG, CO], F32, tag="p9",
                                   name=f"p9_{it}")
                    for g in range(NG):
                        g_ps = psum_g.tile([128, CO], F32, tag="gps",
                                           name=f"g_ps_{it}_{g}")
                        for kb in range(NB):
                            nc.tensor.matmul(
                                g_ps,
                                XB[kb][:, g * 128:(g + 1) * 128],
                                v_sb[kb][:, 0, :],
                                start=(kb == 0),
                                stop=(kb == NB - 1),
                            )
                        nc.vector.scalar_tensor_tensor(
                            out=p9[:, g, :], in0=g_ps, scalar=1.0 / B,
                            in1=WG[:, g, :], op0=ALU.mult, op1=ALU.mult,
                        )
                        if g == 3 or g == 7 or g == 8:
                            lo = 0 if g == 3 else (4 if g == 7 else 8)
                            nc.vector.reduce_sum(
                                Q_all[:, lo * C:(g + 1) * C],
                                p9[:, lo:g + 1, :].rearrange(
                                    "p g (c o) -> p (g c) o", c=C),
                                axis=mybir.AxisListType.X,
                            )
                    agree_ps = psum_misc.tile([RPG, NG * C], F32, tag="agree",
                                              name=f"agree_{it}")
                    nc.tensor.matmul(agree_ps, sel_sb, Q_all,
                                     start=True, stop=True)

                    # ---- exp(b_ij) updated multiplicatively:
                    # exp(b_prev + agree) = exp(b_prev) * exp(agree) ----
                    esr_prev = esr
                    esr = small.tile([RPG, NG * C + NG], F32, tag="esr",
                                     name=f"esr_{it}")
                    if it == 0:
                        nc.scalar.activation(esr[:, :NG * C], agree_ps, ACT.Exp)
                    else:
                        eexp = small.tile([RPG, NG * C], F32, tag="eexp",
                                          name=f"eexp_{it}")
                        nc.scalar.activation(eexp, agree_ps, ACT.Exp)
                        nc.vector.tensor_mul(
                            esr[:, :NG * C], esr_prev[:, :NG * C], eexp
                        )
                    den = small.tile([RPG, NG], F32, tag="sden",
                                     name=f"den_{it}")
                    nc.vector.reduce_sum(
                        den,
                        esr[:, :NG * C].rearrange("p (g c) -> p g c", g=NG),
                        axis=mybir.AxisListType.X,
                    )
                    nc.vector.reciprocal(esr[:, NG * C:], den)
                else:
                    # ---- final iter: ReduceScatter; squash own b-shard ----
                    rs_out = dram.tile([B_SHARD * CO], CC_DT, tag="rs_out")
                    nc.gpsimd.collective_compute(
                        "ReduceScatter", ALU.add, replica_groups=rg,
                        ins=[cc_in.opt()], outs=[rs_out[:]],
                    )
                    s_sb = work.tile([B_SHARD, 1, CO], CC_DT, tag="fs")
                    nc.sync.dma_start(
                        out=s_sb,
                        in_=rs_out.rearrange("(p one n) -> p one n",
                                             n=CO, one=1),
                    )
                    t = work.tile([B_SHARD, 1, CO], F32, tag="ft")
                    bias_b1 = biasb[:B_SHARD, :].rearrange(
                        "p (one co) -> p one co", one=1
                    )
                    nc.vector.scalar_tensor_tensor(
                        out=t, in0=s_sb, scalar=1.0,
                        in1=bias_b1, op0=ALU.mult, op1=ALU.add,
                    )
                    v = _squash(nc, eps_sb, t, B_SHARD, 1, work, tag="f")
                    nc.sync.dma_start(
                        out=y_d[:, :], in_=v.rearrange("p one co -> p (one co)")
                    )

    nc.compile()
    return nc


_NC = None


def kernel(x: np.ndarray, W: np.ndarray, bias: np.ndarray) -> np.ndarray:
    global _NC
    if _NC is None:
        _NC = build()

    x = np.ascontiguousarray(x, dtype=np.float32)
    W = np.ascontiguousarray(W, dtype=np.float32)
    bias = np.ascontiguousarray(bias, dtype=np.float32)

    mm_np = np.float16 if MM_F16 else np.float32
    biasf = bias.reshape(CO)
    sel = np.zeros((128, RPG), dtype=np.float32)
    sel[np.arange(128), np.arange(128) // I] = 1.0
    selT = np.ascontiguousarray(sel.T)

    in_maps = []
    for k in range(N_CORES):
        r0, r1 = k * R_LOC, (k + 1) * R_LOC
        xk = x[:, r0:r1, :].reshape(B, RI_LOC)          # [B,(r,i)]
        wk = W[r0:r1].transpose(0, 3, 1, 2).reshape(RI_LOC, CO)  # [(r,i),(c,o)]
        in_maps.append({
            "xt": np.ascontiguousarray(xk.T.astype(mm_np)),
            "xb": np.ascontiguousarray(xk.astype(mm_np)),
            "wg": np.ascontiguousarray(wk.astype(mm_np)),
            "biasf": biasf,
            "sel": sel,
            "selT": selT,
        })

    global LAST_RESULT
    res = run_bass_kernel_spmd(
        _NC, in_maps, list(range(N_CORES)),
        trace=bool(os.environ.get("BASS_TRACE")),
    )
    LAST_RESULT = res
    v = np.concatenate([res.results[k]["y"] for k in range(N_CORES)], axis=0)
    return v.reshape(B, C, O)[..., None].astype(np.float32)


# revision 3
# speedup vs baseline: 1.1756x; 1.0365x over previous
"""Trainium2 Bass kernel for CapsNet dynamic routing (ClassCapsules).

Reference computation (B=256, R=1152, C=10, O=16, I=8, 3 routing iters):
    u_hat[b,r,c,o] = sum_i W[r,c,o,i] * x[b,r,i]
    b_ij = 0
    for it in 3:
        c_ij = softmax(b_ij, axis=1)                      # over c
        s = sum_r c_ij[r,c] * u_hat[b,r,c,o] + bias       # [B,C,O]
        v = squash(s)
        if it < 2:
            b_ij += mean_b sum_o u_hat[b,r,c,o] v[b,c,o]  # [R,C]
    return v[..., None]

u_hat ([B,R,C,O] = 189MB fp32) is never materialized.  Both routing
contractions are re-associated through the factorization
    s[b,co]    = x~[b,(ri)] @ (c∘W~)[(ri),(co)]
    agree[r,c] = sum_{i,o} W~[(ri),(co)] * G[(ri),(co)],
                 G = (1/B) x~^T v
with x~ = x viewed as [B, R*I] and W~ = W viewed as [R*I, C*O].

Distribution: R is sharded 8 ways (144 r's per core).  Per iteration the
partial s ([256,160], fp16 on the wire) is summed across cores with one
AllReduce; the last iteration uses a ReduceScatter instead and each core
squashes + outputs its own 32-batch shard (2x16 batches in the flat
p-major wire layout).  agree/b_ij/c_ij are fully local to each core's
r-shard.

Matmul operands (x, W, c∘W, v, W∘G) are fp16: 1 cycle/row on the PE and
fast-weight-load; accumulation stays fp32 in PSUM.  The o-reduction of
the agreement is done on the PE as 16 accumulating matmuls over strided
rhs slices of W∘G (sel carries the 1/B), keeping the DVE off the
critical path.
"""

import os
import sys
import types

sys.path.insert(0, "/opt/trn_rl_repo")

# Shim antenv.axon_hooks (absent on this image) so BASS_TRACE=1 profiling
# works through run_bass_kernel_spmd's axon path.  Harmless when unused.
try:
    import antenv.axon_hooks  # noqa: F401
except ImportError:
    try:
        _hooks = types.ModuleType("antenv.axon_hooks")
        _hooks._hook = None
        _hooks.set_axon_ntff_profile_hook = lambda h: setattr(_hooks, "_hook", h)
        _hooks.get_axon_ntff_profile_hook = lambda: _hooks._hook
        sys.modules["antenv.axon_hooks"] = _hooks
        import antenv
        antenv.axon_hooks = _hooks
        from trn_agent_boot.trn_boot import _ntff_profile_via_ctypes
        _hooks.set_axon_ntff_profile_hook(
            _ntff_profile_via_ctypes("/opt/axon/libaxon_pjrt.so")
        )
    except Exception:
        pass

import numpy as np

import concourse.bacc as bacc
import concourse.bass as bass
import concourse.tile as tile
from concourse import mybir
import concourse.bass_utils as _bass_utils
from concourse.bass_utils import run_bass_kernel_spmd

if os.environ.get("BASS_TRACE"):
    _bass_utils.upload_artifacts = lambda tmpdir: ""  # no bucket access here

LAST_RESULT = None

F32 = mybir.dt.float32
F16 = mybir.dt.float16
ALU = mybir.AluOpType
ACT = mybir.ActivationFunctionType

B, R, C, O, I = 256, 1152, 10, 16, 8
CO = C * O                      # 160
N_CORES = 8
R_LOC = R // N_CORES            # 144
RI_LOC = R_LOC * I              # 1152
NG = RI_LOC // 128              # 9 groups of 128 (r,i) rows
NB = B // 128                   # 2 batch partition chunks
B_SHARD = B // N_CORES          # 32 batches output per core
P_SHARD = 128 // N_CORES        # 16 of the 128 b-partitions per core
ITERS = 3
RPG = 128 // I                  # 16 r's per group

CC_F16 = os.environ.get("K_CC_F16", "1") == "1"
MM_F16 = os.environ.get("K_MM_F16", "1") == "1"
CC_DT = F16 if CC_F16 else F32
MM_DT = F16 if MM_F16 else F32
NGP = int(os.environ.get("K_NGP", "6"))   # CW groups on DVE; rest gpsimd


def _squash(nc, eps_sb, t, n_part, nb, pool, tag, out_dt=F32):
    """v = t * n2/((1+n2)*sqrt(n2+eps)); t: [n_part, nb, CO], reduce over o."""
    nc_ = nb * C
    tf = t.rearrange("p nb co -> p (nb co)")
    sq = pool.tile([n_part, nb * CO], F32, tag=f"sq{tag}")
    nc.scalar.square(sq, tf)
    n2 = pool.tile([n_part, nc_], F32, tag=f"n2{tag}")
    nc.vector.reduce_sum(
        n2, sq.rearrange("p (nb c o) -> p nb c o", nb=nb, c=C),
        axis=mybir.AxisListType.X,
    )
    rt = pool.tile([n_part, nc_], F32, tag=f"rt{tag}")
    nc.scalar.activation(rt, n2, ACT.Sqrt, bias=eps_sb[:n_part])
    den = pool.tile([n_part, nc_], F32, tag=f"den{tag}")
    nc.vector.scalar_tensor_tensor(
        out=den, in0=n2, scalar=1.0, in1=rt, op0=ALU.add, op1=ALU.mult,
    )
    rec = pool.tile([n_part, nc_], F32, tag=f"rec{tag}")
    nc.vector.reciprocal(rec, den)
    fac = pool.tile([n_part, nc_], F32, tag=f"fac{tag}")
    nc.vector.tensor_mul(fac, n2, rec)
    v = pool.tile([n_part, nb, CO], out_dt, tag=f"v{tag}")
    fac_b = fac.rearrange(
        "p (nb c one) -> p nb c one", nb=nb, c=C
    ).broadcast_to([n_part, nb, C, O])
    nc.vector.tensor_tensor(
        out=v.rearrange("p nb (c o) -> p nb c o", c=C),
        in0=t.rearrange("p nb (c o) -> p nb c o", c=C),
        in1=fac_b,
        op=ALU.mult,
    )
    return v


def build():
    nc = bacc.Bacc("TRN2", target_bir_lowering=False, debug=False,
                   num_devices=N_CORES)

    # host pre-permutes x~/W~ so every load is one contiguous [128, n] DMA
    xt_d = nc.dram_tensor("xt", [128, NG * B], MM_DT, kind="ExternalInput")
    xb_d = nc.dram_tensor("xb", [B, RI_LOC], MM_DT, kind="ExternalInput")
    wg_d = nc.dram_tensor("wg", [128, NG * CO], MM_DT, kind="ExternalInput")
    bias_d = nc.dram_tensor("biasf", [CO], F32, kind="ExternalInput")
    sel_d = nc.dram_tensor("sel", [128, RPG], MM_DT, kind="ExternalInput")
    selT_d = nc.dram_tensor("selT", [RPG, 128], MM_DT, kind="ExternalInput")
    y_d = nc.dram_tensor("y", [P_SHARD, NB * CO], F32, kind="ExternalOutput")

    rg = [list(range(N_CORES))]

    with tile.TileContext(nc) as tc:
        with (
            tc.tile_pool(name="singles", bufs=1) as singles,
            tc.tile_pool(name="cw_pool", bufs=2) as cw_pool,
            tc.tile_pool(name="work", bufs=2) as work,
            tc.tile_pool(name="small", bufs=3) as small,
            tc.tile_pool(name="psum_s", bufs=1, space="PSUM") as psum_s,
            tc.tile_pool(name="psum_g", bufs=2, space="PSUM") as psum_g,
            tc.tile_pool(name="psum_misc", bufs=1, space="PSUM") as psum_misc,
            tc.tile_pool(name="dram", bufs=2, space="DRAM") as dram,
        ):
            # ---- load inputs (contiguous single DMAs) ----
            XT = singles.tile([128, NG, B], MM_DT)     # x~ [(ri),b] chunked
            nc.sync.dma_start(out=XT, in_=xt_d[:, :])
            WG = singles.tile([128, NG, CO], MM_DT)    # W~ [(ri),(co)] chunked
            nc.sync.dma_start(out=WG, in_=wg_d[:, :])
            XB = []                                    # x [b,(ri)] 2 p-chunks
            for kb in range(NB):
                t = singles.tile([128, RI_LOC], MM_DT, tag=f"xb{kb}",
                                 name=f"xb_sb{kb}")
                nc.sync.dma_start(out=t, in_=xb_d[kb * 128:(kb + 1) * 128, :])
                XB.append(t)
            biasb = singles.tile([128, CO], F32)
            nc.sync.dma_start(
                out=biasb,
                in_=bass.AP(tensor=bias_d, offset=0, ap=[[0, 128], [1, CO]]),
            )
            sel_sb = singles.tile([128, RPG], MM_DT)   # one-hot ri->r, * 1/B
            nc.sync.dma_start(out=sel_sb, in_=sel_d[:, :])
            selT_sb = singles.tile([RPG, 128], MM_DT)  # one-hot r->ri
            nc.sync.dma_start(out=selT_sb, in_=selT_d[:, :])

            eps_sb = singles.tile([128, 1], F32)
            nc.vector.memset(eps_sb, 1e-8)

            esr_e = None   # [16, 90] fp32: exp(b_ij), multiplicative state

            for it in range(ITERS):
                # ---- CW = c∘W~ (it>0); it=0 uses uniform c=0.1 folded later
                if it == 0:
                    CW = WG
                else:
                    # esr2 = softmax(b) rows as f16, broadcast 16->128 via PE
                    cp_ps = psum_misc.tile([128, NG * C], F32, tag="cp",
                                           name=f"cp_ps_{it}")
                    nc.tensor.matmul(cp_ps, selT_sb, esr2, start=True,
                                     stop=True)
                    cp_sb = small.tile([128, NG * C], F16, tag="cpart",
                                       name=f"cp_sb_{it}")
                    nc.scalar.copy(cp_sb, cp_ps)
                    CW = cw_pool.tile([128, NG, CO], MM_DT, tag="cw",
                                      name=f"cw_{it}")
                    for g in range(NG):
                        c_b = cp_sb[:, g * C:(g + 1) * C].rearrange(
                            "p (c one) -> p c one", one=1
                        ).broadcast_to([128, C, O])
                        eng = nc.vector if g < NGP else nc.gpsimd
                        eng.tensor_tensor(
                            out=CW[:, g, :].rearrange("p (c o) -> p c o", c=C),
                            in0=WG[:, g, :].rearrange("p (c o) -> p c o", c=C),
                            in1=c_b, op=ALU.mult,
                        )

                # ---- s partial: [256,160] = x~^T @ CW, K = (ri) local ----
                s_ps = [psum_s.tile([128, CO], F32, tag=f"s{kb}",
                                    name=f"s_ps{kb}_{it}")
                        for kb in range(NB)]
                for kb in range(NB):
                    for g in range(NG):
                        nc.tensor.matmul(
                            s_ps[kb],
                            XT[:, g, kb * 128:(kb + 1) * 128],
                            CW[:, g, :],
                            start=(g == 0),
                            stop=(g == NG - 1),
                        )

                # stage both b-chunks p-major and ship with one DMA
                cc_in = dram.tile([128, NB * CO], CC_DT, tag="cc_in",
                                  name=f"cc_in_{it}")
                s_stage = work.tile([128, NB, CO], CC_DT, tag="sstage",
                                    name=f"s_stage_{it}")
                for kb in range(NB):
                    nc.scalar.copy(s_stage[:, kb, :], s_ps[kb])
                nc.sync.dma_start(
                    out=cc_in[:, :],
                    in_=s_stage.rearrange("p nb co -> p (nb co)"),
                )

                if it < ITERS - 1:
                    # ---- AllReduce s; every core squashes the full batch
                    cc_out = dram.tile([128, NB * CO], CC_DT, tag="cc_out",
                                       name=f"cc_out_{it}")
                    nc.gpsimd.collective_compute(
                        "AllReduce", ALU.add, replica_groups=rg,
                        ins=[cc_in.opt()], outs=[cc_out.opt()],
                    )
                    s_sb = work.tile([128, NB, CO], CC_DT, tag="ssb",
                                     name=f"s_sb_{it}")
                    nc.sync.dma_start(
                        out=s_sb.rearrange("p nb co -> p (nb co)"),
                        in_=cc_out[:, :],
                    )
                    # per-kb squash so kb=0's G matmuls overlap kb=1's squash
                    v_sb = []
                    bias_b1 = biasb.rearrange("p (one co) -> p one co", one=1)
                    for kb in range(NB):
                        t = work.tile([128, 1, CO], F32, tag=f"t{kb}",
                                      name=f"t{kb}_{it}")
                        nc.vector.scalar_tensor_tensor(
                            out=t, in0=s_sb[:, kb:kb + 1, :],
                            scalar=(0.1 if it == 0 else 1.0),
                            in1=bias_b1, op0=ALU.mult, op1=ALU.add,
                        )
                        v_sb.append(_squash(nc, eps_sb, t, 128, 1, work,
                                            tag=str(kb), out_dt=MM_DT))

                    # ---- G = x~^T v ; agree = (1/B) sum_io W∘G via PE ----
                    p9 = work.tile([128, NG, C, O], MM_DT, tag="p9",
                                   name=f"p9_{it}")
                    for g in range(NG):
                        g_ps = psum_g.tile([128, CO], F32, tag="gps",
                                           name=f"g_ps_{it}_{g}")
                        for kb in range(NB):
                            nc.tensor.matmul(
                                g_ps,
                                XB[kb][:, g * 128:(g + 1) * 128],
                                v_sb[kb][:, 0, :],
                                start=(kb == 0),
                                stop=(kb == NB - 1),
                            )
                        nc.vector.tensor_tensor(
                            out=p9[:, g, :, :],
                            in0=g_ps.rearrange("p (c o) -> p c o", c=C),
                            in1=WG[:, g, :].rearrange("p (c o) -> p c o", c=C),
                            op=ALU.mult,
                        )
                    # o-reduction on the PE: 16 accumulating matmuls over
                    # strided rhs slices; sel carries the 1/B mean factor.
                    agree_ps = psum_misc.tile([RPG, NG * C], F32, tag="agree",
                                              name=f"agree_{it}")
                    for o in range(O):
                        nc.tensor.matmul(
                            agree_ps, sel_sb, p9[:, :, :, o],
                            start=(o == 0), stop=(o == O - 1),
                        )

                    # ---- exp(b_ij) updated multiplicatively:
                    # exp(b_prev + agree) = exp(b_prev) * exp(agree) ----
                    esr_prev = esr_e
                    esr_e = small.tile([RPG, NG * C], F32, tag="esr",
                                       name=f"esr_{it}")
                    if it == 0:
                        nc.scalar.activation(esr_e, agree_ps, ACT.Exp)
                    else:
                        eexp = small.tile([RPG, NG * C], F32, tag="eexp",
                                          name=f"eexp_{it}")
                        nc.scalar.activation(eexp, agree_ps, ACT.Exp)
                        nc.vector.tensor_mul(esr_e, esr_prev, eexp)
                    den = small.tile([RPG, NG], F32, tag="sden",
                                     name=f"den_{it}")
                    nc.vector.reduce_sum(
                        den,
                        esr_e.rearrange("p (g c) -> p g c", g=NG),
                        axis=mybir.AxisListType.X,
                    )
                    rec9 = small.tile([RPG, NG], F32, tag="srec",
                                      name=f"rec_{it}")
                    nc.vector.reciprocal(rec9, den)
                    esr2 = small.tile([RPG, NG * C], F16, tag="esr2",
                                      name=f"esr2_{it}")
                    rec_b = rec9.rearrange(
                        "p (g one) -> p g one", one=1
                    ).broadcast_to([RPG, NG, C])
                    nc.vector.tensor_tensor(
                        out=esr2.rearrange("p (g c) -> p g c", g=NG),
                        in0=esr_e.rearrange("p (g c) -> p g c", g=NG),
                        in1=rec_b, op=ALU.mult,
                    )
                else:
                    # ---- final iter: ReduceScatter; squash own b-shard ----
                    # flat p-major wire layout: this core's chunk is
                    # partitions [16k,16k+16) x [NB,CO]
                    rs_out = dram.tile([P_SHARD * NB * CO], CC_DT,
                                       tag="rs_out")
                    nc.gpsimd.collective_compute(
                        "ReduceScatter", ALU.add, replica_groups=rg,
                        ins=[cc_in.opt()], outs=[rs_out[:]],
                    )
                    s_f = work.tile([P_SHARD, NB, CO], CC_DT, tag="fs")
                    nc.sync.dma_start(
                        out=s_f,
                        in_=rs_out.rearrange("(p nb co) -> p nb co",
                                             nb=NB, co=CO),
                    )
                    t = work.tile([P_SHARD, NB, CO], F32, tag="ft")
                    bias_b2 = biasb[:P_SHARD, :].rearrange(
                        "p (one co) -> p one co", one=1
                    ).broadcast_to([P_SHARD, NB, CO])
                    nc.vector.scalar_tensor_tensor(
                        out=t, in0=s_f, scalar=1.0,
                        in1=bias_b2, op0=ALU.mult, op1=ALU.add,
                    )
                    v = _squash(nc, eps_sb, t, P_SHARD, NB, work, tag="f")
                    nc.sync.dma_start(
                        out=y_d[:, :],
                        in_=v.rearrange("p nb co -> p (nb co)"),
                    )

    nc.compile()
    return nc


_NC = None


def kernel(x: np.ndarray, W: np.ndarray, bias: np.ndarray) -> np.ndarray:
    global _NC
    if _NC is None:
        _NC = build()

    x = np.ascontiguousarray(x, dtype=np.float32)
    W = np.ascontiguousarray(W, dtype=np.float32)
    bias = np.ascontiguousarray(bias, dtype=np.float32)

    mm_np = np.float16 if MM_F16 else np.float32
    biasf = bias.reshape(CO)
    sel = np.zeros((128, RPG), dtype=np.float32)
    sel[np.arange(128), np.arange(128) // I] = 1.0 / B
    selT = np.zeros((RPG, 128), dtype=np.float32)
    selT[np.arange(128) // I, np.arange(128)] = 1.0
    sel = sel.astype(mm_np)
    selT = np.ascontiguousarray(selT.astype(mm_np))

    in_maps = []
    for k in range(N_CORES):
        r0, r1 = k * R_LOC, (k + 1) * R_LOC
        xk = x[:, r0:r1, :].reshape(B, RI_LOC)          # [B,(r,i)]
        wk = W[r0:r1].transpose(0, 3, 1, 2).reshape(RI_LOC, CO)  # [(r,i),(c,o)]
        # pre-permute [(g p), n] -> [p, (g n)] so the SBUF load is one
        # fully-contiguous DMA
        xtk = np.ascontiguousarray(
            xk.T.astype(mm_np).reshape(NG, 128, B).transpose(1, 0, 2)
            .reshape(128, NG * B)
        )
        wgk = np.ascontiguousarray(
            wk.astype(mm_np).reshape(NG, 128, CO).transpose(1, 0, 2)
            .reshape(128, NG * CO)
        )
        in_maps.append({
            "xt": xtk,
            "xb": np.ascontiguousarray(xk.astype(mm_np)),
            "wg": wgk,
            "biasf": biasf,
            "sel": sel,
            "selT": selT,
        })

    global LAST_RESULT
    res = run_bass_kernel_spmd(
        _NC, in_maps, list(range(N_CORES)),
        trace=bool(os.environ.get("BASS_TRACE")),
    )
    LAST_RESULT = res
    # y_k[p, (nb co)] holds batches b = nb*128 + 16k + p
    out = np.empty((B, CO), dtype=np.float32)
    for k in range(N_CORES):
        yk = res.results[k]["y"].reshape(P_SHARD, NB, CO)
        for nb in range(NB):
            out[nb * 128 + P_SHARD * k:nb * 128 + P_SHARD * (k + 1), :] = (
                yk[:, nb, :]
            )
    return out.reshape(B, C, O)[..., None].astype(np.float32)


# revision 10
# speedup vs baseline: 1.1760x; 1.0004x over previous
"""Trainium2 Bass kernel for CapsNet dynamic routing (ClassCapsules).

Reference computation (B=256, R=1152, C=10, O=16, I=8, 3 routing iters):
    u_hat[b,r,c,o] = sum_i W[r,c,o,i] * x[b,r,i]
    b_ij = 0
    for it in 3:
        c_ij = softmax(b_ij, axis=1)                      # over c
        s = sum_r c_ij[r,c] * u_hat[b,r,c,o] + bias       # [B,C,O]
        v = squash(s)
        if it < 2:
            b_ij += mean_b sum_o u_hat[b,r,c,o] v[b,c,o]  # [R,C]
    return v[..., None]

u_hat ([B,R,C,O] = 189MB fp32) is never materialized.  Both routing
contractions are re-associated through the factorization
    s[b,co]    = x~[b,(ri)] @ (c∘W~)[(ri),(co)]
    agree[r,c] = sum_{i,o} W~[(ri),(co)] * G[(ri),(co)],
                 G = (1/B) x~^T v
with x~ = x viewed as [B, R*I] and W~ = W viewed as [R*I, C*O].

Distribution: R is sharded 8 ways (144 r's per core).  Per iteration the
partial s ([256,160], fp16 on the wire) is summed across cores with one
AllReduce; the last iteration uses a ReduceScatter instead and each core
squashes + outputs its own 32-batch shard (2x16 batches in the flat
p-major wire layout).  agree/b_ij/c_ij are fully local to each core's
r-shard.

Matmul operands (x, W, c∘W, v, W∘G) are fp16: 1 cycle/row on the PE and
fast-weight-load; accumulation stays fp32 in PSUM.  The o-reduction of
the agreement is done on the PE as 16 accumulating matmuls over strided
rhs slices of W∘G (sel carries the 1/B), keeping the DVE off the
critical path.
"""

import os
import sys
import types

sys.path.insert(0, "/opt/trn_rl_repo")

# Shim antenv.axon_hooks (absent on this image) so BASS_TRACE=1 profiling
# works through run_bass_kernel_spmd's axon path.  Harmless when unused.
try:
    import antenv.axon_hooks  # noqa: F401
except ImportError:
    try:
        _hooks = types.ModuleType("antenv.axon_hooks")
        _hooks._hook = None
        _hooks.set_axon_ntff_profile_hook = lambda h: setattr(_hooks, "_hook", h)
        _hooks.get_axon_ntff_profile_hook = lambda: _hooks._hook
        sys.modules["antenv.axon_hooks"] = _hooks
        import antenv
        antenv.axon_hooks = _hooks
        from trn_agent_boot.trn_boot import _ntff_profile_via_ctypes
        _hooks.set_axon_ntff_profile_hook(
            _ntff_profile_via_ctypes("/opt/axon/libaxon_pjrt.so")
        )
    except Exception:
        pass

import numpy as np

import concourse.bacc as bacc
import concourse.bass as bass
import concourse.tile as tile
from concourse import mybir
import concourse.bass_utils as _bass_utils
from concourse.bass_utils import run_bass_kernel_spmd

if os.environ.get("BASS_TRACE"):
    _bass_utils.upload_artifacts = lambda tmpdir: ""  # no bucket access here

LAST_RESULT = None

F32 = mybir.dt.float32
F16 = mybir.dt.float16
ALU = mybir.AluOpType
ACT = mybir.ActivationFunctionType

B, R, C, O, I = 256, 1152, 10, 16, 8
CO = C * O                      # 160
N_CORES = 8
R_LOC = R // N_CORES            # 144
RI_LOC = R_LOC * I              # 1152
NG = RI_LOC // 128              # 9 groups of 128 (r,i) rows
NB = B // 128                   # 2 batch partition chunks
B_SHARD = B // N_CORES          # 32 batches output per core
P_SHARD = 128 // N_CORES        # 16 of the 128 b-partitions per core
ITERS = 3
RPG = 128 // I                  # 16 r's per group

CC_F16 = os.environ.get("K_CC_F16", "1") == "1"
MM_F16 = os.environ.get("K_MM_F16", "1") == "1"
CC_DT = F16 if CC_F16 else F32
MM_DT = F16 if MM_F16 else F32
NGP = int(os.environ.get("K_NGP", "5"))   # CW groups on DVE; rest gpsimd


def _squash(nc, eps_sb, t, n_part, nb, pool, tag, out_dt=F32):
    """v = t * n2/((1+n2)*sqrt(n2+eps)); t: [n_part, nb, CO], reduce over o."""
    nc_ = nb * C
    tf = t.rearrange("p nb co -> p (nb co)")
    sq = pool.tile([n_part, nb * CO], F32, tag=f"sq{tag}")
    nc.scalar.square(sq, tf)
    n2 = pool.tile([n_part, nc_], F32, tag=f"n2{tag}")
    nc.vector.reduce_sum(
        n2, sq.rearrange("p (nb c o) -> p nb c o", nb=nb, c=C),
        axis=mybir.AxisListType.X,
    )
    rt = pool.tile([n_part, nc_], F32, tag=f"rt{tag}")
    nc.scalar.activation(rt, n2, ACT.Sqrt, bias=eps_sb[:n_part])
    den = pool.tile([n_part, nc_], F32, tag=f"den{tag}")
    nc.vector.scalar_tensor_tensor(
        out=den, in0=n2, scalar=1.0, in1=rt, op0=ALU.add, op1=ALU.mult,
    )
    rec = pool.tile([n_part, nc_], F32, tag=f"rec{tag}")
    nc.vector.reciprocal(rec, den)
    fac = pool.tile([n_part, nc_], F32, tag=f"fac{tag}")
    nc.vector.tensor_mul(fac, n2, rec)
    v = pool.tile([n_part, nb, CO], out_dt, tag=f"v{tag}")
    fac_b = fac.rearrange(
        "p (nb c one) -> p nb c one", nb=nb, c=C
    ).broadcast_to([n_part, nb, C, O])
    nc.vector.tensor_tensor(
        out=v.rearrange("p nb (c o) -> p nb c o", c=C),
        in0=t.rearrange("p nb (c o) -> p nb c o", c=C),
        in1=fac_b,
        op=ALU.mult,
    )
    return v


def build():
    nc = bacc.Bacc("TRN2", target_bir_lowering=False, debug=False,
                   num_devices=N_CORES)

    # host pre-permutes x~/W~ so every load is one contiguous [128, n] DMA
    xt_d = nc.dram_tensor("xt", [128, NG * B], MM_DT, kind="ExternalInput")
    xb_d = nc.dram_tensor("xb", [B, RI_LOC], MM_DT, kind="ExternalInput")
    wg_d = nc.dram_tensor("wg", [128, NG * CO], MM_DT, kind="ExternalInput")
    bias_d = nc.dram_tensor("biasf", [CO], F32, kind="ExternalInput")
    sel_d = nc.dram_tensor("sel", [128, RPG], MM_DT, kind="ExternalInput")
    selT_d = nc.dram_tensor("selT", [RPG, 128], MM_DT, kind="ExternalInput")
    y_d = nc.dram_tensor("y", [P_SHARD, NB * CO], F32, kind="ExternalOutput")

    rg = [list(range(N_CORES))]

    with tile.TileContext(nc) as tc:
        with (
            tc.tile_pool(name="singles", bufs=1) as singles,
            tc.tile_pool(name="cw_pool", bufs=2) as cw_pool,
            tc.tile_pool(name="work", bufs=2) as work,
            tc.tile_pool(name="small", bufs=3) as small,
            tc.tile_pool(name="psum_s", bufs=1, space="PSUM") as psum_s,
            tc.tile_pool(name="psum_g", bufs=4, space="PSUM") as psum_g,
            tc.tile_pool(name="psum_misc", bufs=1, space="PSUM") as psum_misc,
            tc.tile_pool(name="dram", bufs=2, space="DRAM") as dram,
        ):
            # ---- load inputs (contiguous DMAs, split across the two
            # HWDGE-capable issue queues so issue time halves) ----
            XT = singles.tile([128, NG, B], MM_DT)     # x~ [(ri),b] chunked
            nc.sync.dma_start(out=XT, in_=xt_d[:, :])
            WG = singles.tile([128, NG, CO], MM_DT)    # W~ [(ri),(co)] chunked
            nc.scalar.dma_start(out=WG, in_=wg_d[:, :])
            XB = []                                    # x [b,(ri)] 2 p-chunks
            for kb in range(NB):
                t = singles.tile([128, RI_LOC], MM_DT, tag=f"xb{kb}",
                                 name=f"xb_sb{kb}")
                eng = nc.sync if kb == 0 else nc.scalar
                eng.dma_start(out=t, in_=xb_d[kb * 128:(kb + 1) * 128, :])
                XB.append(t)
            biasb = singles.tile([128, CO], F32)
            nc.sync.dma_start(
                out=biasb,
                in_=bass.AP(tensor=bias_d, offset=0, ap=[[0, 128], [1, CO]]),
            )
            sel_sb = singles.tile([128, RPG], MM_DT)   # one-hot ri->r, * 1/B
            nc.scalar.dma_start(out=sel_sb, in_=sel_d[:, :])
            selT_sb = singles.tile([RPG, 128], MM_DT)  # one-hot r->ri
            nc.sync.dma_start(out=selT_sb, in_=selT_d[:, :])

            eps_sb = singles.tile([128, 1], F32)
            nc.vector.memset(eps_sb, 1e-8)

            esr_e = None   # [16, 90] fp32: exp(b_ij), multiplicative state

            for it in range(ITERS):
                # ---- CW = c∘W~ (it>0); it=0 uses uniform c=0.1 folded later
                if it == 0:
                    CW = WG
                else:
                    # esr2 = softmax(b) rows as f16, broadcast 16->128 via PE
                    cp_ps = psum_misc.tile([128, NG * C], F32, tag="cp",
                                           name=f"cp_ps_{it}")
                    nc.tensor.matmul(cp_ps, selT_sb, esr2, start=True,
                                     stop=True)
                    cp_sb = small.tile([128, NG * C], F16, tag="cpart",
                                       name=f"cp_sb_{it}")
                    nc.scalar.copy(cp_sb, cp_ps)
                    CW = cw_pool.tile([128, NG, CO], MM_DT, tag="cw",
                                      name=f"cw_{it}")
                    for g in range(NG):
                        c_b = cp_sb[:, g * C:(g + 1) * C].rearrange(
                            "p (c one) -> p c one", one=1
                        ).broadcast_to([128, C, O])
                        eng = nc.vector if g < NGP else nc.gpsimd
                        eng.tensor_tensor(
                            out=CW[:, g, :].rearrange("p (c o) -> p c o", c=C),
                            in0=WG[:, g, :].rearrange("p (c o) -> p c o", c=C),
                            in1=c_b, op=ALU.mult,
                        )

                # ---- s partial: [256,160] = x~^T @ CW, K = (ri) local ----
                s_ps = [psum_s.tile([128, CO], F32, tag=f"s{kb}",
                                    name=f"s_ps{kb}_{it}")
                        for kb in range(NB)]
                for kb in range(NB):
                    for g in range(NG):
                        nc.tensor.matmul(
                            s_ps[kb],
                            XT[:, g, kb * 128:(kb + 1) * 128],
                            CW[:, g, :],
                            start=(g == 0),
                            stop=(g == NG - 1),
                        )

                # stage both b-chunks p-major and ship with one DMA
                cc_in = dram.tile([128, NB * CO], CC_DT, tag="cc_in",
                                  name=f"cc_in_{it}")
                s_stage = work.tile([128, NB, CO], CC_DT, tag="sstage",
                                    name=f"s_stage_{it}")
                for kb in range(NB):
                    nc.scalar.copy(s_stage[:, kb, :], s_ps[kb])
                nc.sync.dma_start(
                    out=cc_in[:, :],
                    in_=s_stage.rearrange("p nb co -> p (nb co)"),
                )
                # pull the Sqrt ACT-table load into the collective window:
                # this read of s_stage schedules right after staging, and the
                # squash sqrt then hits a warm table
                dsq = small.tile([1, 1], F32, tag="dsq", name=f"dsq_{it}")
                nc.scalar.activation(dsq, s_stage[:1, 0, :1], ACT.Sqrt,
                                     bias=eps_sb[:1], scale=0.0)

                if it < ITERS - 1:
                    # ---- AllReduce s; every core squashes the full batch
                    cc_out = dram.tile([128, NB * CO], CC_DT, tag="cc_out",
                                       name=f"cc_out_{it}")
                    nc.gpsimd.collective_compute(
                        "AllReduce", ALU.add, replica_groups=rg,
                        ins=[cc_in.opt()], outs=[cc_out.opt()],
                    )
                    s_sb = work.tile([128, NB, CO], CC_DT, tag="ssb",
                                     name=f"s_sb_{it}")
                    nc.sync.dma_start(
                        out=s_sb.rearrange("p nb co -> p (nb co)"),
                        in_=cc_out[:, :],
                    )
                    # per-kb squash so kb=0's G matmuls overlap kb=1's squash
                    v_sb = []
                    bias_b1 = biasb.rearrange("p (one co) -> p one co", one=1)
                    for kb in range(NB):
                        t = work.tile([128, 1, CO], F32, tag=f"t{kb}",
                                      name=f"t{kb}_{it}")
                        if it == 0:
                            # uniform c=0.1 folded in here; STT is DVE-only
                            nc.vector.scalar_tensor_tensor(
                                out=t, in0=s_sb[:, kb:kb + 1, :],
                                scalar=0.1,
                                in1=bias_b1, op0=ALU.mult, op1=ALU.add,
                            )
                        else:
                            nc.gpsimd.tensor_tensor(
                                out=t, in0=s_sb[:, kb:kb + 1, :],
                                in1=bias_b1, op=ALU.add,
                            )
                        v_sb.append(_squash(nc, eps_sb, t, 128, 1, work,
                                            tag=str(kb), out_dt=MM_DT))
                    # pull the Exp ACT-table load into the G-matmul window
                    dex = small.tile([1, 1], F32, tag="dex", name=f"dex_{it}")
                    nc.scalar.activation(dex, v_sb[0][:1, 0, :1], ACT.Exp,
                                         scale=0.0)

                    # ---- G = x~^T v ; agree = (1/B) sum_io W∘G via PE ----
                    # p9 is o-major so each agree matmul reads a contiguous
                    # [128, NG*C] rhs slice
                    p9 = work.tile([128, O, NG * C], MM_DT, tag="p9",
                                   name=f"p9_{it}")
                    for g in range(NG):
                        g_ps = psum_g.tile([128, CO], F32, tag="gps",
                                           name=f"g_ps_{it}_{g}")
                        for kb in range(NB):
                            nc.tensor.matmul(
                                g_ps,
                                XB[kb][:, g * 128:(g + 1) * 128],
                                v_sb[kb][:, 0, :],
                                start=(kb == 0),
                                stop=(kb == NB - 1),
                            )
                        nc.vector.tensor_tensor(
                            out=p9[:, :, g * C:(g + 1) * C],
                            in0=g_ps.rearrange("p (c o) -> p o c", c=C),
                            in1=WG[:, g, :].rearrange("p (c o) -> p o c", c=C),
                            op=ALU.mult,
                        )
                    # o-reduction on the PE: 16 accumulating matmuls;
                    # sel carries the 1/B mean factor.
                    agree_ps = psum_misc.tile([RPG, NG * C], F32, tag="agree",
                                              name=f"agree_{it}")
                    for o in range(O):
                        nc.tensor.matmul(
                            agree_ps, sel_sb, p9[:, o, :],
                            start=(o == 0), stop=(o == O - 1),
                        )

                    # ---- exp(b_ij) updated multiplicatively:
                    # exp(b_prev + agree) = exp(b_prev) * exp(agree) ----
                    esr_prev = esr_e
                    esr_e = small.tile([RPG, NG * C], F32, tag="esr",
                                       name=f"esr_{it}")
                    if it == 0:
                        nc.scalar.activation(esr_e, agree_ps, ACT.Exp)
                    else:
                        eexp = small.tile([RPG, NG * C], F32, tag="eexp",
                                          name=f"eexp_{it}")
                        nc.scalar.activation(eexp, agree_ps, ACT.Exp)
                        nc.vector.tensor_mul(esr_e, esr_prev, eexp)
                    den = small.tile([RPG, NG], F32, tag="sden",
                                     name=f"den_{it}")
                    nc.vector.reduce_sum(
                        den,
                        esr_e.rearrange("p (g c) -> p g c", g=NG),
                        axis=mybir.AxisListType.X,
                    )
                    rec9 = small.tile([RPG, NG], F32, tag="srec",
                                      name=f"rec_{it}")
                    nc.vector.reciprocal(rec9, den)
                    esr2 = small.tile([RPG, NG * C], F16, tag="esr2",
                                      name=f"esr2_{it}")
                    rec_b = rec9.rearrange(
                        "p (g one) -> p g one", one=1
                    ).broadcast_to([RPG, NG, C])
                    nc.vector.tensor_tensor(
                        out=esr2.rearrange("p (g c) -> p g c", g=NG),
                        in0=esr_e.rearrange("p (g c) -> p g c", g=NG),
                        in1=rec_b, op=ALU.mult,
                    )
                else:
                    # ---- final iter: ReduceScatter; squash own b-shard ----
                    # flat p-major wire layout: this core's chunk is
                    # partitions [16k,16k+16) x [NB,CO]
                    rs_out = dram.tile([P_SHARD * NB * CO], CC_DT,
                                       tag="rs_out")
                    nc.gpsimd.collective_compute(
                        "ReduceScatter", ALU.add, replica_groups=rg,
                        ins=[cc_in.opt()], outs=[rs_out[:]],
                    )
                    s_f = work.tile([P_SHARD, NB, CO], CC_DT, tag="fs")
                    nc.sync.dma_start(
                        out=s_f,
                        in_=rs_out.rearrange("(p nb co) -> p nb co",
                                             nb=NB, co=CO),
                    )
                    t = work.tile([P_SHARD, NB, CO], F32, tag="ft")
                    bias_b2 = biasb[:P_SHARD, :].rearrange(
                        "p (one co) -> p one co", one=1
                    ).broadcast_to([P_SHARD, NB, CO])
                    nc.vector.scalar_tensor_tensor(
                        out=t, in0=s_f, scalar=1.0,
                        in1=bias_b2, op0=ALU.mult, op1=ALU.add,
                    )
                    v = _squash(nc, eps_sb, t, P_SHARD, NB, work, tag="f")
                    nc.sync.dma_start(
                        out=y_d[:, :],
                        in_=v.rearrange("p nb co -> p (nb co)"),
                    )

    nc.compile()
    return nc


_NC = None


def kernel(x: np.ndarray, W: np.ndarray, bias: np.ndarray) -> np.ndarray:
    global _NC
    if _NC is None:
        _NC = build()

    x = np.ascontiguousarray(x, dtype=np.float32)
    W = np.ascontiguousarray(W, dtype=np.float32)
    bias = np.ascontiguousarray(bias, dtype=np.float32)

    mm_np = np.float16 if MM_F16 else np.float32
    biasf = bias.reshape(CO)
    sel = np.zeros((128, RPG), dtype=np.float32)
    sel[np.arange(128), np.arange(128) // I] = 1.0 / B
    selT = np.zeros((RPG, 128), dtype=np.float32)
    selT[np.arange(128) // I, np.arange(128)] = 1.0
    sel = sel.astype(mm_np)
    selT = np.ascontiguousarray(selT.astype(mm_np))

    in_maps = []
    for k in range(N_CORES):
        r0, r1 = k * R_LOC, (k + 1) * R_LOC
        xk = x[:, r0:r1, :].reshape(B, RI_LOC)          # [B,(r,i)]
        wk = W[r0:r1].transpose(0, 3, 1, 2).reshape(RI_LOC, CO)  # [(r,i),(c,o)]
        # pre-permute [(g p), n] -> [p, (g n)] so the SBUF load is one
        # fully-contiguous DMA
        xtk = np.ascontiguousarray(
            xk.T.astype(mm_np).reshape(NG, 128, B).transpose(1, 0, 2)
            .reshape(128, NG * B)
        )
        wgk = np.ascontiguousarray(
            wk.astype(mm_np).reshape(NG, 128, CO).transpose(1, 0, 2)
            .reshape(128, NG * CO)
        )
        in_maps.append({
            "xt": xtk,
            "xb": np.ascontiguousarray(xk.astype(mm_np)),
            "wg": wgk,
            "biasf": biasf,
            "sel": sel,
            "selT": selT,
        })

    global LAST_RESULT
    res = run_bass_kernel_spmd(
        _NC, in_maps, list(range(N_CORES)),
        trace=bool(os.environ.get("BASS_TRACE")),
    )
    LAST_RESULT = res
    # y_k[p, (nb co)] holds batches b = nb*128 + 16k + p
    out = np.empty((B, CO), dtype=np.float32)
    for k in range(N_CORES):
        yk = res.results[k]["y"].reshape(P_SHARD, NB, CO)
        for nb in range(NB):
            out[nb * 128 + P_SHARD * k:nb * 128 + P_SHARD * (k + 1), :] = (
                yk[:, nb, :]
            )
    return out.reshape(B, C, O)[..., None].astype(np.float32)


# revision 18
# speedup vs baseline: 1.2609x; 1.0722x over previous
"""Trainium2 Bass kernel for CapsNet dynamic routing (ClassCapsules).

Reference computation (B=256, R=1152, C=10, O=16, I=8, 3 routing iters):
    u_hat[b,r,c,o] = sum_i W[r,c,o,i] * x[b,r,i]
    b_ij = 0
    for it in 3:
        c_ij = softmax(b_ij, axis=1)                      # over c
        s = sum_r c_ij[r,c] * u_hat[b,r,c,o] + bias       # [B,C,O]
        v = squash(s)
        if it < 2:
            b_ij += mean_b sum_o u_hat[b,r,c,o] v[b,c,o]  # [R,C]
    return v[..., None]

u_hat ([B,R,C,O] = 189MB fp32) is never materialized.  Both routing
contractions are re-associated through the factorization
    s[b,co]    = x~[b,(ri)] @ (c∘W~)[(ri),(co)]
    agree[r,c] = sum_{i,o} W~[(ri),(co)] * G[(ri),(co)],
                 G = (1/B) x~^T v
with x~ = x viewed as [B, R*I] and W~ = W viewed as [R*I, C*O].

Distribution: R is sharded 8 ways (144 r's per core).  Per iteration the
partial s ([256,160], fp16 on the wire) is summed across cores with one
AllReduce; the last iteration uses a ReduceScatter instead and each core
squashes + outputs its own 32-batch shard (2x16 batches in the flat
p-major wire layout).  agree/b_ij/c_ij are fully local to each core's
r-shard.

Matmul operands (x, W, c∘W, v, W∘G) are fp16: 1 cycle/row on the PE and
fast-weight-load; accumulation stays fp32 in PSUM.  The o-reduction of
the agreement is done on the PE as 16 accumulating matmuls over strided
rhs slices of W∘G (sel carries the 1/B), keeping the DVE off the
critical path.
"""

import os
import sys
import types

sys.path.insert(0, "/opt/trn_rl_repo")

# Shim antenv.axon_hooks (absent on this image) so BASS_TRACE=1 profiling
# works through run_bass_kernel_spmd's axon path.  Harmless when unused.
try:
    import antenv.axon_hooks  # noqa: F401
except ImportError:
    try:
        _hooks = types.ModuleType("antenv.axon_hooks")
        _hooks._hook = None
        _hooks.set_axon_ntff_profile_hook = lambda h: setattr(_hooks, "_hook", h)
        _hooks.get_axon_ntff_profile_hook = lambda: _hooks._hook
        sys.modules["antenv.axon_hooks"] = _hooks
        import antenv
        antenv.axon_hooks = _hooks
        from trn_agent_boot.trn_boot import _ntff_profile_via_ctypes
        _hooks.set_axon_ntff_profile_hook(
            _ntff_profile_via_ctypes("/opt/axon/libaxon_pjrt.so")
        )
    except Exception:
        pass

import numpy as np

import concourse.bacc as bacc
import concourse.bass as bass
import concourse.tile as tile
from concourse import mybir
import concourse.bass_utils as _bass_utils
from concourse.bass_utils import run_bass_kernel_spmd

if os.environ.get("BASS_TRACE"):
    _bass_utils.upload_artifacts = lambda tmpdir: ""  # no bucket access here

LAST_RESULT = None

F32 = mybir.dt.float32
F16 = mybir.dt.float16
ALU = mybir.AluOpType
ACT = mybir.ActivationFunctionType

B, R, C, O, I = 256, 1152, 10, 16, 8
CO = C * O                      # 160
N_CORES = 8
R_LOC = R // N_CORES            # 144
RI_LOC = R_LOC * I              # 1152
NG = RI_LOC // 128              # 9 groups of 128 (r,i) rows
NB = B // 128                   # 2 batch partition chunks
B_SHARD = B // N_CORES          # 32 batches output per core
P_SHARD = 128 // N_CORES        # 16 of the 128 b-partitions per core
ITERS = 3
RPG = 128 // I                  # 16 r's per group

CC_F16 = os.environ.get("K_CC_F16", "1") == "1"
MM_F16 = os.environ.get("K_MM_F16", "1") == "1"
CC_DT = F16 if CC_F16 else F32
MM_DT = F16 if MM_F16 else F32
NGP = int(os.environ.get("K_NGP", "4"))   # CW groups on DVE; rest gpsimd


def _squash(nc, eps_sb, t, n_part, nb, pool, tag, out_dt=F32, act_sq=True):
    """v = t * n2/((1+n2)*sqrt(n2+eps)); t: [n_part, nb, CO] o-major,
    reduce over o."""
    nc_ = nb * C
    tf = t.rearrange("p nb co -> p (nb co)")
    sq = pool.tile([n_part, nb * CO], F32, tag=f"sq{tag}")
    if act_sq:
        nc.scalar.square(sq, tf)
    else:
        nc.vector.tensor_mul(sq, tf, tf)
    n2 = pool.tile([n_part, nc_], F32, tag=f"n2{tag}")
    nc.vector.reduce_sum(
        n2, sq.rearrange("p (nb o c) -> p nb c o", nb=nb, c=C),
        axis=mybir.AxisListType.X,
    )
    rt = pool.tile([n_part, nc_], F32, tag=f"rt{tag}")
    nc.scalar.activation(rt, n2, ACT.Sqrt, bias=eps_sb[:n_part])
    den = pool.tile([n_part, nc_], F32, tag=f"den{tag}")
    nc.vector.scalar_tensor_tensor(
        out=den, in0=n2, scalar=1.0, in1=rt, op0=ALU.add, op1=ALU.mult,
    )
    rec = pool.tile([n_part, nc_], F32, tag=f"rec{tag}")
    nc.vector.reciprocal(rec, den)
    fac = pool.tile([n_part, nc_], F32, tag=f"fac{tag}")
    nc.vector.tensor_mul(fac, n2, rec)
    v = pool.tile([n_part, nb, CO], out_dt, tag=f"v{tag}")
    fac_b = fac.rearrange(
        "p (nb c one) -> p nb one c", nb=nb, c=C
    ).broadcast_to([n_part, nb, O, C])
    nc.vector.tensor_tensor(
        out=v.rearrange("p nb (o c) -> p nb o c", c=C),
        in0=t.rearrange("p nb (o c) -> p nb o c", c=C),
        in1=fac_b,
        op=ALU.mult,
    )
    return v


def build():
    nc = bacc.Bacc("TRN2", target_bir_lowering=False, debug=False,
                   num_devices=N_CORES)

    # host pre-permutes x~/W~ so every load is one contiguous [128, n] DMA
    xt_d = nc.dram_tensor("xt", [128, NG * B], MM_DT, kind="ExternalInput")
    xb_d = nc.dram_tensor("xb", [B, RI_LOC], MM_DT, kind="ExternalInput")
    wg_d = nc.dram_tensor("wg", [128, NG * CO], MM_DT, kind="ExternalInput")
    bias_d = nc.dram_tensor("biasf", [CO], F32, kind="ExternalInput")
    sel_d = nc.dram_tensor("sel", [128, RPG], MM_DT, kind="ExternalInput")
    selT_d = nc.dram_tensor("selT", [RPG, 128], MM_DT, kind="ExternalInput")
    y_d = nc.dram_tensor("y", [P_SHARD, NB * CO], F32, kind="ExternalOutput")

    rg = [list(range(N_CORES))]

    with tile.TileContext(nc) as tc:
        with (
            tc.tile_pool(name="singles", bufs=1) as singles,
            tc.tile_pool(name="cw_pool", bufs=2) as cw_pool,
            tc.tile_pool(name="work", bufs=2) as work,
            tc.tile_pool(name="small", bufs=3) as small,
            tc.tile_pool(name="psum_s", bufs=1, space="PSUM") as psum_s,
            tc.tile_pool(name="psum_g", bufs=4, space="PSUM") as psum_g,
            tc.tile_pool(name="psum_misc", bufs=1, space="PSUM") as psum_misc,
            tc.tile_pool(name="dram", bufs=2, space="DRAM") as dram,
        ):
            # ---- load inputs (contiguous DMAs, split across the two
            # HWDGE-capable issue queues so issue time halves) ----
            XT = singles.tile([128, NG, B], MM_DT)     # x~ [(ri),b] chunked
            nc.sync.dma_start(out=XT, in_=xt_d[:, :])
            WG = singles.tile([128, NG, CO], MM_DT)    # W~ [(ri),(co)] chunked
            nc.scalar.dma_start(out=WG, in_=wg_d[:, :])
            XB = []                                    # x [b,(ri)] 2 p-chunks
            for kb in range(NB):
                t = singles.tile([128, RI_LOC], MM_DT, tag=f"xb{kb}",
                                 name=f"xb_sb{kb}")
                eng = nc.sync if kb == 0 else nc.scalar
                eng.dma_start(out=t, in_=xb_d[kb * 128:(kb + 1) * 128, :])
                XB.append(t)
            biasb = singles.tile([128, CO], F32)
            nc.sync.dma_start(
                out=biasb,
                in_=bass.AP(tensor=bias_d, offset=0, ap=[[0, 128], [1, CO]]),
            )
            sel_sb = singles.tile([128, RPG], MM_DT)   # one-hot ri->r, * 1/B
            nc.scalar.dma_start(out=sel_sb, in_=sel_d[:, :])
            selT_sb = singles.tile([RPG, 128], MM_DT)  # one-hot r->ri
            nc.sync.dma_start(out=selT_sb, in_=selT_d[:, :])

            eps_sb = singles.tile([128, 1], F32)
            nc.vector.memset(eps_sb, 1e-8)

            esr_e = None   # [16, 90] fp32: exp(b_ij), multiplicative state

            for it in range(ITERS):
                # ---- CW = c∘W~ (it>0); it=0 uses uniform c=0.1 folded later
                if it == 0:
                    CW = WG
                else:
                    # esr2 = softmax(b) rows as f16, broadcast 16->128 via PE
                    cp_ps = psum_misc.tile([128, NG * C], F32, tag="cp",
                                           name=f"cp_ps_{it}")
                    nc.tensor.matmul(cp_ps, selT_sb, esr2, start=True,
                                     stop=True)
                    cp_sb = small.tile([128, NG * C], F16, tag="cpart",
                                       name=f"cp_sb_{it}")
                    nc.scalar.copy(cp_sb, cp_ps)
                    CW = cw_pool.tile([128, NG, CO], MM_DT, tag="cw",
                                      name=f"cw_{it}")
                    for g in range(NG):
                        c_b = cp_sb[:, g * C:(g + 1) * C].rearrange(
                            "p (one c) -> p one c", one=1
                        ).broadcast_to([128, O, C])
                        eng = nc.vector if g < NGP else nc.gpsimd
                        eng.tensor_tensor(
                            out=CW[:, g, :].rearrange("p (o c) -> p o c", c=C),
                            in0=WG[:, g, :].rearrange("p (o c) -> p o c", c=C),
                            in1=c_b, op=ALU.mult,
                        )

                # ---- s partial: [256,160] = x~^T @ CW, K = (ri) local ----
                s_ps = [psum_s.tile([128, CO], F32, tag=f"s{kb}",
                                    name=f"s_ps{kb}_{it}")
                        for kb in range(NB)]
                for kb in range(NB):
                    for g in range(NG):
                        nc.tensor.matmul(
                            s_ps[kb],
                            XT[:, g, kb * 128:(kb + 1) * 128],
                            CW[:, g, :],
                            start=(g == 0),
                            stop=(g == NG - 1),
                        )

                # stage both b-chunks p-major and ship with one DMA
                cc_in = dram.tile([128, NB * CO], CC_DT, tag="cc_in",
                                  name=f"cc_in_{it}")
                s_stage = work.tile([128, NB, CO], CC_DT, tag="sstage",
                                    name=f"s_stage_{it}")
                nc.scalar.copy(s_stage[:, 0, :], s_ps[0])
                nc.vector.tensor_copy(s_stage[:, 1, :], s_ps[1])
                nc.sync.dma_start(
                    out=cc_in[:, :],
                    in_=s_stage.rearrange("p nb co -> p (nb co)"),
                )
                # pull the Sqrt ACT-table load into the collective window:
                # this read of s_stage schedules right after staging, and the
                # squash sqrt then hits a warm table
                dsq = small.tile([1, 1], F32, tag="dsq", name=f"dsq_{it}")
                nc.scalar.activation(dsq, s_stage[:1, 0, :1], ACT.Sqrt,
                                     bias=eps_sb[:1], scale=0.0)

                if it < ITERS - 1:
                    # ---- AllReduce s; every core squashes the full batch
                    cc_out = dram.tile([128, NB * CO], CC_DT, tag="cc_out",
                                       name=f"cc_out_{it}")
                    nc.gpsimd.collective_compute(
                        "AllReduce", ALU.add, replica_groups=rg,
                        ins=[cc_in.opt()], outs=[cc_out.opt()],
                    )
                    s_sb = work.tile([128, NB, CO], CC_DT, tag="ssb",
                                     name=f"s_sb_{it}")
                    nc.sync.dma_start(
                        out=s_sb.rearrange("p nb co -> p (nb co)"),
                        in_=cc_out[:, :],
                    )
                    # per-kb squash so kb=0's G matmuls overlap kb=1's squash
                    v_sb = []
                    bias_b1 = biasb.rearrange("p (one co) -> p one co", one=1)
                    for kb in range(NB):
                        t = work.tile([128, 1, CO], F32, tag=f"t{kb}",
                                      name=f"t{kb}_{it}")
                        if it == 0:
                            # uniform c=0.1 folded in here; STT is DVE-only
                            nc.vector.scalar_tensor_tensor(
                                out=t, in0=s_sb[:, kb:kb + 1, :],
                                scalar=0.1,
                                in1=bias_b1, op0=ALU.mult, op1=ALU.add,
                            )
                        else:
                            nc.gpsimd.tensor_tensor(
                                out=t, in0=s_sb[:, kb:kb + 1, :],
                                in1=bias_b1, op=ALU.add,
                            )
                        v_sb.append(_squash(nc, eps_sb, t, 128, 1, work,
                                            tag=str(kb), out_dt=MM_DT))
                    # pull the Exp ACT-table load into the G-matmul window
                    dex = small.tile([1, 1], F32, tag="dex", name=f"dex_{it}")
                    nc.scalar.activation(dex, v_sb[0][:1, 0, :1], ACT.Exp,
                                         scale=0.0)

                    # ---- G = x~^T v ; agree = (1/B) sum_io W∘G via PE ----
                    # everything is o-major, so the p9 = W∘G products are
                    # fully contiguous; the agree matmuls read per-o slices
                    # with contiguous 10-element runs
                    p9 = work.tile([128, NG, O, C], MM_DT, tag="p9",
                                   name=f"p9_{it}")
                    for g in range(NG):
                        g_ps = psum_g.tile([128, CO], F32, tag="gps",
                                           name=f"g_ps_{it}_{g}")
                        for kb in range(NB):
                            nc.tensor.matmul(
                                g_ps,
                                XB[kb][:, g * 128:(g + 1) * 128],
                                v_sb[kb][:, 0, :],
                                start=(kb == 0),
                                stop=(kb == NB - 1),
                            )
                        nc.vector.tensor_tensor(
                            out=p9[:, g, :, :],
                            in0=g_ps.rearrange("p (o c) -> p o c", c=C),
                            in1=WG[:, g, :].rearrange("p (o c) -> p o c", c=C),
                            op=ALU.mult,
                        )
                    # o-reduction on the PE: 16 accumulating matmuls;
                    # sel carries the 1/B mean factor.
                    agree_ps = psum_misc.tile([RPG, NG * C], F32, tag="agree",
                                              name=f"agree_{it}")
                    for o in range(O):
                        nc.tensor.matmul(
                            agree_ps, sel_sb, p9[:, :, o, :],
                            start=(o == 0), stop=(o == O - 1),
                        )

                    # ---- exp(b_ij) updated multiplicatively:
                    # exp(b_prev + agree) = exp(b_prev) * exp(agree) ----
                    esr_prev = esr_e
                    esr_e = small.tile([RPG, NG * C], F32, tag="esr",
                                       name=f"esr_{it}")
                    if it == 0:
                        nc.scalar.activation(esr_e, agree_ps, ACT.Exp)
                    else:
                        eexp = small.tile([RPG, NG * C], F32, tag="eexp",
                                          name=f"eexp_{it}")
                        nc.scalar.activation(eexp, agree_ps, ACT.Exp)
                        nc.vector.tensor_mul(esr_e, esr_prev, eexp)
                    den = small.tile([RPG, NG], F32, tag="sden",
                                     name=f"den_{it}")
                    nc.vector.reduce_sum(
                        den,
                        esr_e.rearrange("p (g c) -> p g c", g=NG),
                        axis=mybir.AxisListType.X,
                    )
                    rec9 = small.tile([RPG, NG], F32, tag="srec",
                                      name=f"rec_{it}")
                    nc.vector.reciprocal(rec9, den)
                    esr2 = small.tile([RPG, NG * C], F16, tag="esr2",
                                      name=f"esr2_{it}")
                    rec_b = rec9.rearrange(
                        "p (g one) -> p g one", one=1
                    ).broadcast_to([RPG, NG, C])
                    nc.vector.tensor_tensor(
                        out=esr2.rearrange("p (g c) -> p g c", g=NG),
                        in0=esr_e.rearrange("p (g c) -> p g c", g=NG),
                        in1=rec_b, op=ALU.mult,
                    )
                else:
                    # ---- final iter: ReduceScatter; squash own b-shard ----
                    # flat p-major wire layout: this core's chunk is
                    # partitions [16k,16k+16) x [NB,CO]
                    rs_out = dram.tile([P_SHARD * NB * CO], CC_DT,
                                       tag="rs_out")
                    nc.gpsimd.collective_compute(
                        "ReduceScatter", ALU.add, replica_groups=rg,
                        ins=[cc_in.opt()], outs=[rs_out[:]],
                    )
                    s_f = work.tile([P_SHARD, NB, CO], CC_DT, tag="fs")
                    nc.sync.dma_start(
                        out=s_f,
                        in_=rs_out.rearrange("(p nb co) -> p nb co",
                                             nb=NB, co=CO),
                    )
                    t = work.tile([P_SHARD, NB, CO], F32, tag="ft")
                    bias_b2 = biasb[:P_SHARD, :].rearrange(
                        "p (one co) -> p one co", one=1
                    ).broadcast_to([P_SHARD, NB, CO])
                    nc.vector.scalar_tensor_tensor(
                        out=t, in0=s_f, scalar=1.0,
                        in1=bias_b2, op0=ALU.mult, op1=ALU.add,
                    )
                    v = _squash(nc, eps_sb, t, P_SHARD, NB, work, tag="f",
                                act_sq=False)
                    nc.sync.dma_start(
                        out=y_d[:, :],
                        in_=v.rearrange("p nb co -> p (nb co)"),
                    )

    nc.compile()
    return nc


_NC = None


def kernel(x: np.ndarray, W: np.ndarray, bias: np.ndarray) -> np.ndarray:
    global _NC
    if _NC is None:
        _NC = build()

    x = np.ascontiguousarray(x, dtype=np.float32)
    W = np.ascontiguousarray(W, dtype=np.float32)
    bias = np.ascontiguousarray(bias, dtype=np.float32)

    mm_np = np.float16 if MM_F16 else np.float32
    biasf = np.ascontiguousarray(bias.T).reshape(CO)   # o-major
    sel = np.zeros((128, RPG), dtype=np.float32)
    sel[np.arange(128), np.arange(128) // I] = 1.0 / B
    selT = np.zeros((RPG, 128), dtype=np.float32)
    selT[np.arange(128) // I, np.arange(128)] = 1.0
    sel = sel.astype(mm_np)
    selT = np.ascontiguousarray(selT.astype(mm_np))

    in_maps = []
    for k in range(N_CORES):
        r0, r1 = k * R_LOC, (k + 1) * R_LOC
        xk = x[:, r0:r1, :].reshape(B, RI_LOC)          # [B,(r,i)]
        wk = W[r0:r1].transpose(0, 3, 2, 1).reshape(RI_LOC, CO)  # [(r,i),(o,c)]
        # pre-permute [(g p), n] -> [p, (g n)] so the SBUF load is one
        # fully-contiguous DMA
        xtk = np.ascontiguousarray(
            xk.T.astype(mm_np).reshape(NG, 128, B).transpose(1, 0, 2)
            .reshape(128, NG * B)
        )
        wgk = np.ascontiguousarray(
            wk.astype(mm_np).reshape(NG, 128, CO).transpose(1, 0, 2)
            .reshape(128, NG * CO)
        )
        in_maps.append({
            "xt": xtk,
            "xb": np.ascontiguousarray(xk.astype(mm_np)),
            "wg": wgk,
            "biasf": biasf,
            "sel": sel,
            "selT": selT,
        })

    global LAST_RESULT
    res = run_bass_kernel_spmd(
        _NC, in_maps, list(range(N_CORES)),
        trace=bool(os.environ.get("BASS_TRACE")),
    )
    LAST_RESULT = res
    # y_k[p, (nb o c)] holds batches b = nb*128 + 16k + p, o-major
    out = np.empty((B, C, O), dtype=np.float32)
    for k in range(N_CORES):
        yk = res.results[k]["y"].reshape(P_SHARD, NB, O, C).transpose(
            0, 1, 3, 2)
        for nb in range(NB):
            out[nb * 128 + P_SHARD * k:nb * 128 + P_SHARD * (k + 1)] = (
                yk[:, nb]
            )
    return out[..., None].astype(np.float32)
